# revision 1
# baseline (speedup 1.0000x reference)
"""Trainium2 Bass kernel for CenterDependentPool2D (v3).

Input  x: (8, 64, 448, 448) fp32  ->  Output: (8, 64, 224, 224) fp32.

Per core = one batch element.  Partition p = c + 64*wg: channel c, wg 0 =
out cols 0..111 (natural j), wg 1 = out cols 223..112 (MIRRORED local j).
The mirror is applied by the Activation-engine fp32->fp16 casts (strided /
reversed reads are free there), so every DVE op is a unified 128-partition
instruction and each ring occupies a single low-j column interval =>
per-band column gating of the whole pyramid.

Five ring windows (k in {2,8,14,20,26}, stride 2, reflect pad == clip)
decompose over pair arrays E[i]=max(x[2i],x[2i+1]), O[i]=max(x[2i+1],
x[2i+2]) in both dims.  32-row out bands (amortize the ~0.5us DVE drain
per instruction): Act casts de-interleaved column-parity arrays, DVE
builds pair maxes + shifted-max doubling pyramids (fp16 tensor_tensor,
2x mode), ring combines and blend (nested-disk copy_predicated) are
column-gated to ring bounding boxes; output stored fp16, upcast on host.
"""

import numpy as np

import concourse.bass as bass
import concourse.mybir as mybir
from concourse.tile import TileContext
from concourse.bass_utils import run_bass_kernel_spmd

# ---------------- problem constants ----------------
B, C, IN, OUT = 8, 64, 448, 224
CEN = 112
OW = 112
EW = 124          # pair-array width
WIN = 250         # input chunk cols (incl pads)
NEG = -30000.0
RADII = (60, 75, 90, 105)
DT = mybir.dt.float16
MX = mybir.AluOpType.max

# out-row bands: [0,24), [24,56), ..., [184,216), [216,224)
BANDS = [(0, 24)] + [(24 + 32 * k, 56 + 32 * k) for k in range(6)] \
    + [(216, 224)]
NBANDS = len(BANDS)

# ---------------- static geometry ----------------

_yy, _xx = np.mgrid[0:OUT, 0:OUT]
_D2 = (_yy - CEN) ** 2 + (_xx - CEN) ** 2
NESTED = np.stack([(_D2 < R * R) for R in RADII])
RING_ID = 4 - NESTED.sum(0)


def _localize(a):
    return a[:, 0:CEN], a[:, ::-1][:, 0:CEN]


def _hull(a, b):
    if a is None:
        return b
    if b is None:
        return a
    return (min(a[0], b[0]), max(a[1], b[1]))


class BandGeom:
    def __init__(self, it):
        self.it = it
        y0, y1 = BANDS[it]
        self.y0, self.y1, self.H = y0, y1, y1 - y0
        r0, r1 = _localize(RING_ID)
        rows = slice(y0, y1)
        self.ring = []
        for r in range(5):
            m = (r0[rows] == r) | (r1[rows] == r)
            if not m.any():
                self.ring.append(None)
                continue
            ridx = np.where(m.any(1))[0]
            cidx = np.where(m.any(0))[0]
            self.ring.append(dict(
                rlo=y0 + int(ridx.min()), rhi=y0 + int(ridx.max()) + 1,
                clo=int(cidx.min()), chi=int(cidx.max()) + 1))
        assert self.ring[4] is not None and self.ring[4]["clo"] == 0
        assert self.ring[4]["rlo"] == y0 and self.ring[4]["rhi"] == y1
        self.b4 = self.ring[4]["chi"]
        # rings 1-3: split tall blend bboxes into 2 row-groups with tight
        # per-group col bboxes (annuli sweep diagonally; the full bbox is
        # mostly empty mask)
        self.groups = {}
        for r in (1, 2, 3):
            g0 = self.ring[r]
            if g0 is None:
                continue
            full = [(g0["rlo"], g0["rhi"], g0["clo"], g0["chi"])]
            nr = g0["rhi"] - g0["rlo"]
            m = (r0[rows] == r) | (r1[rows] == r)

            def mk(nparts):
                gl = []
                bounds = [g0["rlo"] + nr * k // nparts
                          for k in range(nparts + 1)]
                for ra, rb in zip(bounds, bounds[1:]):
                    sub = m[ra - y0:rb - y0]
                    if not sub.any():
                        continue
                    ri = np.where(sub.any(1))[0]
                    ci = np.where(sub.any(0))[0]
                    gl.append((ra + int(ri.min()), ra + int(ri.max()) + 1,
                               int(ci.min()), int(ci.max()) + 1))
                return gl

            def cost(gl):
                # elems at CP rate + per-op drain (in 1.04ns elem units)
                return sum((b - a) * (d - c) for a, b, c, d in gl)                     + len(gl) * 530
            cands = [full]
            if nr >= 16:
                cands.append(mk(2))
            if nr >= 32:
                cands.append(mk(4))
            self.groups[r] = min(cands, key=cost)
        # ring0 inscribed square: cols where every ring0-bbox row is inside
        # disk60 for both wg variants (unconditional copy, no mask needed)
        self.sq0 = None
        if self.ring[0] is not None:
            g0 = self.ring[0]
            n0, n1 = _localize(NESTED[0])
            rs = slice(g0["rlo"], g0["rhi"])
            allin = n0[rs].all(0) & n1[rs].all(0)
            ci = np.where(allin)[0]
            if len(ci) and ci.max() - ci.min() >= 8:
                self.sq0 = (int(ci.min()), int(ci.max()) + 1)
                assert self.sq0[0] >= g0["clo"] and self.sq0[1] <= g0["chi"]

    def blend_mask(self, r, box=None):
        g = self.ring[r]
        if box is None:
            box = (g["rlo"], g["rhi"], g["clo"], g["chi"])
        rlo, rhi, clo, chi = box
        n0, n1 = _localize(RING_ID == r)
        s0 = n0[rlo:rhi, clo:chi].astype(np.uint8)
        s1 = n1[rlo:rhi, clo:chi].astype(np.uint8)
        m = np.zeros((128,) + s0.shape, np.uint8)
        m[0:64] = s0[None]
        m[64:128] = s1[None]
        return m


class Extents:
    """Backward-propagated (rows, cols) per pyramid level; rows in E/O-row
    (== out-row) space, cols in pair-e space, half-open."""

    def __init__(self, g):
        y0, y1 = g.y0, g.y1
        s13_r, s13_c = (y0 - 6, y1 - 6), (0, g.b4)
        v13_r, v13_c = s13_r, (s13_c[0], s13_c[1] + 5)
        s8_r, s8_c = (v13_r[0], v13_r[1] + 5), v13_c
        a8_r, a8_c = s8_r, (s8_c[0], s8_c[1] + 4)
        r2 = g.ring[2]
        if r2 is not None:
            s7_r = (r2["rlo"] - 3, r2["rhi"] - 3)
            s7_c = (r2["clo"] + 3, r2["chi"] + 3)
            u7_r, u7_c = s7_r, (s7_c[0], s7_c[1] + 3)
            s4_r = _hull((a8_r[0], a8_r[1] + 4), (u7_r[0], u7_r[1] + 3))
            s4_c = _hull(a8_c, u7_c)
        else:
            s7_r = s7_c = u7_r = u7_c = None
            s4_r, s4_c = (a8_r[0], a8_r[1] + 4), a8_c
        a4_r, a4_c = s4_r, (s4_c[0], s4_c[1] + 2)
        s2_r, s2_c = (a4_r[0], a4_r[1] + 2), a4_c
        a2_r, a2_c = s2_r, (s2_c[0], s2_c[1] + 1)
        self.ee_rows = (a2_r[0], a2_r[1] + 1)

        r1, r3 = g.ring[1], g.ring[3]
        s10_r = s10_c = w10_r = w10_c = None
        s8o_r = s8o_c = a8o_r = a8o_c = None
        s4o_r = s4o_c = None
        if r1 is not None:
            s4o_r = (r1["rlo"] - 2, r1["rhi"] - 2)
            s4o_c = (r1["clo"] + 4, r1["chi"] + 4)
        if r3 is not None:
            s10_r = (r3["rlo"] - 5, r3["rhi"] - 5)
            s10_c = (r3["clo"] + 1, r3["chi"] + 1)
            w10_r, w10_c = s10_r, (s10_c[0], s10_c[1] + 2)
            s8o_r, s8o_c = (w10_r[0], w10_r[1] + 2), w10_c
            a8o_r, a8o_c = s8o_r, (s8o_c[0], s8o_c[1] + 4)
            s4o_r = _hull(s4o_r, (a8o_r[0], a8o_r[1] + 4))
            s4o_c = _hull(s4o_c, a8o_c)
        if s4o_r is not None:
            a4o_r, a4o_c = s4o_r, (s4o_c[0], s4o_c[1] + 2)
            s2o_r, s2o_c = (a4o_r[0], a4o_r[1] + 2), a4o_c
            a2o_r, a2o_c = s2o_r, (s2o_c[0], s2o_c[1] + 1)
            self.oo_rows = (a2o_r[0], a2o_r[1] + 1)
            self.oo_cols = (a2o_c[0], a2o_c[1] + 1)
        else:
            a4o_r = a4o_c = s2o_r = s2o_c = a2o_r = a2o_c = None
            self.oo_rows = None
            self.oo_cols = None

        self.lv = dict(
            a2=(a2_r, a2_c), s2=(s2_r, s2_c), a4=(a4_r, a4_c),
            s4=(s4_r, s4_c), a8=(a8_r, a8_c), s8=(s8_r, s8_c),
            u7=(u7_r, u7_c), s7=(s7_r, s7_c), v13=(v13_r, v13_c),
            s13=(s13_r, s13_c),
            a2o=(a2o_r, a2o_c), s2o=(s2o_r, s2o_c), a4o=(a4o_r, a4o_c),
            s4o=(s4o_r, s4o_c), a8o=(a8o_r, a8o_c), s8o=(s8o_r, s8o_c),
            w10=(w10_r, w10_c), s10=(s10_r, s10_c),
        )


GEOMS = [BandGeom(it) for it in range(NBANDS)]
EXTENTS = [Extents(g) for g in GEOMS]

# columns the O-side arrays must carry per band: this band's pyramid needs
# union next band's (carry rows serve it)
OWG = []
for _it in range(NBANDS):
    _a = EXTENTS[_it].oo_cols
    _b = EXTENTS[_it + 1].oo_cols if _it + 1 < NBANDS else None
    _u = _hull(_a, _b)
    OWG.append(_u if _u is not None else (0, 124))

for _g, _e in zip(GEOMS, EXTENTS):
    _ob = 32 * _g.it - 14
    assert _e.ee_rows[0] >= _ob and _e.ee_rows[1] <= _ob + 46
    if _e.oo_rows is not None:
        assert _e.oo_rows[0] >= _ob and _e.oo_rows[1] <= _ob + 46
    for _n, (_rr, _cc) in _e.lv.items():
        if _cc is not None:
            assert 0 <= _cc[0] and _cc[1] <= 125, (_g.it, _n, _cc)

# pooled level tiles: tag sharing by disjoint lifetime
LV_TAG = dict(a2="tP", a4="tP", a8="tP", v13="tP",
              s2="tQ", s8="tQ", s4="tS4", u7="tT", w10="tT",
              s7="tS7", a2o="tPo", a4o="tPo", a8o="tPo",
              s2o="tQo", s8o="tQo", s4o="tS4o", s10="tS10")
TAG_MAX = {}
for _e in EXTENTS:
    for _n, (_rr, _cc) in _e.lv.items():
        if _n == "s13" or _rr is None:
            continue
        t = LV_TAG[_n]
        sz = TAG_MAX.get(t, (0, 0))
        TAG_MAX[t] = (max(sz[0], _rr[1] - _rr[0]),
                      max(sz[1], _cc[1] - _cc[0]))


def _build_strips():
    blobs, bands, offs = [], [], []
    pos = 0
    for g in GEOMS:
        start = pos
        ent = {}
        for r in (3, 2, 1, 0):
            if g.ring[r] is None:
                continue
            if r in (1, 2, 3):
                for gi, box in enumerate(g.groups[r]):
                    m = g.blend_mask(r, box)
                    nr, nc = m.shape[1], m.shape[2]
                    ent[(r, gi)] = (pos - start, nr, nc)
                    blobs.append(np.ascontiguousarray(m).reshape(128, -1))
                    pos += nr * nc
                continue
            m = g.blend_mask(r)
            nr, nc = m.shape[1], m.shape[2]
            if r == 0 and g.sq0 is not None:
                clo = g.ring[0]["clo"]
                ja, jb = g.sq0
                for key, mm in (("0L", m[:, :, 0:ja - clo]),
                                ("0R", m[:, :, jb - clo:])):
                    if mm.shape[2] == 0:
                        continue
                    ent[key] = (pos - start, nr, mm.shape[2])
                    blobs.append(np.ascontiguousarray(mm).reshape(128, -1))
                    pos += nr * mm.shape[2]
                continue
            ent[r] = (pos - start, nr, nc)
            blobs.append(m.reshape(128, -1))
            pos += nr * nc
        offs.append(ent)
        bands.append((start, pos - start))
    blob = (np.concatenate(blobs, 1) if blobs
            else np.zeros((128, 1), np.uint8))
    return blob, bands, offs


STRIP_BLOB, STRIP_BANDS, STRIP_OFFS = _build_strips()
STRIP_MAX = max(sz for _, sz in STRIP_BANDS)

BLEND_SRC = {3: ("s10", -5, 1), 2: ("s7", -3, 3),
             1: ("s4o", -2, 4), 0: ("ee", 0, 6)}

# ---------------- shared band program ----------------


def _emit_band(be, it):
    g, E = GEOMS[it], EXTENTS[it]
    y0, y1, H, b4 = g.y0, g.y1, g.H, g.b4
    OB = 32 * it - 14
    ee, oo, Ew, Ow, ol = be.ee, be.oo, be.Ew, be.Ow, be.owlast

    glo, ghi = OWG[it]
    slo, shi = EXTENTS[it].oo_cols if EXTENTS[it].oo_cols else (0, 1)
    if 0 < it < 7:
        az = be.azeo()
        be.dma_in(it, 0, az)
        be.dma_in(it, 1, az)
        be.max2(Ew[:, :, :], az[:, :, 0:124], az[:, :, 126:250])
        # Ow rows 0:37 serve only this band (own cols); rows 37:64 also
        # feed the next band's carry (2-band union cols)
        olo, ohi = EXTENTS[it].oo_cols
        be.max2(Ow[:, 0:37, olo:ohi], az[:, 0:37, 126 + olo:126 + ohi],
                az[:, 0:37, 1 + olo:1 + ohi])
        be.max2(Ow[:, 37:64, glo:ghi], az[:, 37:64, 126 + glo:126 + ghi],
                az[:, 37:64, 1 + glo:1 + ghi])
    if it > 0:
        be.gcopy(ee[:, 0:14, :], ee[:, 32:46, :])
        be.gcopy(oo[:, 0:14, :], oo[:, 32:46, :])
        if it == 7:
            be.memset(ee[:, 14:46, :], NEG)
            be.memset(oo[:, 14:46, :], NEG)
            be.scopy(oo[:, 13:14, slo:shi], ol[:, 0:1, slo:shi])
        else:
            be.max2(oo[:, 13:14, slo:shi], ol[:, 0:1, slo:shi],
                    Ow[:, 0:1, slo:shi])
            be.max2(ee[:, 14:46, :], Ew[:, 0:64:2, :], Ew[:, 1:64:2, :])
            olo, ohi = EXTENTS[it].oo_cols
            be.max2(oo[:, 14:32, olo:ohi],
                    Ow[:, 1:36:2, olo:ohi], Ow[:, 2:37:2, olo:ohi])
            be.max2(oo[:, 32:45, glo:ghi],
                    Ow[:, 37:62:2, glo:ghi], Ow[:, 38:63:2, glo:ghi])
            be.acopy(ol[:, 0:1, glo:ghi], Ow[:, 63:64, glo:ghi])

    # ---- pyramids ----
    P = {"ee": (ee, OB, 0), "oo": (oo, OB, 0)}
    out16 = be.out16()

    def comb(name, src, d, axis, dst=None, rows=None, cols=None,
             keep=False):
        rr, cc = E.lv[name]
        if rows is not None:
            rr = rows
        if cols is not None:
            cc = cols
        nr, nc = rr[1] - rr[0], cc[1] - cc[0]
        st, sr0, sc0 = P[src]
        ra, rb = rr[0] - sr0, rr[1] - sr0
        ca, cb = cc[0] - sc0, cc[1] - sc0
        assert ra >= 0 and ca >= 0, (it, name)
        if axis == "r":
            a = st[:, ra:rb, ca:cb]
            b = st[:, ra + d:rb + d, ca:cb]
        else:
            a = st[:, ra:rb, ca:cb]
            b = st[:, ra:rb, ca + d:cb + d]
        if dst is None:
            if keep:
                t, tr0, tc0 = P[name]
                be.max2(t[:, rr[0] - tr0:rr[1] - tr0,
                          cc[0] - tc0:cc[1] - tc0], a, b)
            else:
                t = be.lv(name)
                be.max2(t[:, 0:nr, 0:nc], a, b)
                P[name] = (t, rr[0], cc[0])
        else:
            be.max2(dst, a, b)
            P[name] = None

    def prep(name):
        rr, cc = E.lv[name]
        P[name] = (be.lv(name), rr[0], cc[0])

    def gcomb(name, src, d, axis, roff, c0off, c1off, groups):
        """Per-blend-group combine into one shared tile (groups have
        disjoint row ranges)."""
        if len(groups) == 1:
            comb(name, src, d, axis)
            return
        prep(name)
        for (grlo, grhi, gclo, gchi) in groups:
            comb(name, src, d, axis, rows=(grlo + roff, grhi + roff),
                 cols=(gclo + c0off, gchi + c1off), keep=True)

    if it == 0:
        # fast start: chunk-0 work (E/O rows 0..15) ordered ahead of
        # chunk-1-dependent ops so the DVE runs during the second DMA
        az = be.azeo()
        be.dma_rows(az, 0, 16)
        be.dma_rows(az, 16, 32)
        be.dma_in(it, 1, az)
        be.max2(Ew[:, 0:16, :], az[:, 0:16, 0:124], az[:, 0:16, 126:250])
        be.max2(Ow[:, 0:16, glo:ghi], az[:, 0:16, 126 + glo:126 + ghi],
                az[:, 0:16, 1 + glo:1 + ghi])
        be.max2(Ew[:, 16:32, :], az[:, 16:32, 0:124], az[:, 16:32, 126:250])
        be.max2(Ow[:, 16:32, glo:ghi], az[:, 16:32, 126 + glo:126 + ghi],
                az[:, 16:32, 1 + glo:1 + ghi])
        be.memset(ee[:, 0:14, :], NEG)
        be.memset(oo[:, 0:14, :], NEG)
        be.scopy(oo[:, 13:14, slo:shi], Ow[:, 0:1, slo:shi])
        be.max2(ee[:, 14:30, :], Ew[:, 0:32:2, :], Ew[:, 1:32:2, :])
        be.max2(oo[:, 14:29, glo:ghi],
                Ow[:, 1:31:2, glo:ghi], Ow[:, 2:32:2, glo:ghi])
        a2rr, _ = E.lv["a2"]
        s2rr, _ = E.lv["s2"]
        mid = 9
        comb("a2", "ee", 1, "r", rows=(a2rr[0], mid))
        comb("s2", "a2", 1, "c", rows=(s2rr[0], mid))
        be.max2(Ew[:, 32:64, :], az[:, 32:64, 0:124], az[:, 32:64, 126:250])
        be.max2(Ow[:, 32:64, glo:ghi], az[:, 32:64, 126 + glo:126 + ghi],
                az[:, 32:64, 1 + glo:1 + ghi])
        be.max2(ee[:, 30:46, :], Ew[:, 32:64:2, :], Ew[:, 33:64:2, :])
        be.max2(oo[:, 29:45, glo:ghi],
                Ow[:, 31:63:2, glo:ghi], Ow[:, 32:64:2, glo:ghi])
        be.acopy(ol[:, 0:1, glo:ghi], Ow[:, 63:64, glo:ghi])
        comb("a2", "ee", 1, "r", rows=(mid, a2rr[1]), keep=True)
        comb("s2", "a2", 1, "c", rows=(mid, s2rr[1]), keep=True)
    else:
        comb("a2", "ee", 1, "r")
        comb("s2", "a2", 1, "c")
    comb("a4", "s2", 2, "r")
    comb("s4", "a4", 2, "c")
    comb("a8", "s4", 4, "r")
    comb("s8", "a8", 4, "c")
    comb("v13", "s8", 5, "r")
    comb("s13", "v13", 5, "c", dst=out16[:, 0:H, 0:b4])
    if g.ring[2] is not None:
        gcomb("u7", "s4", 3, "r", -3, 3, 6, g.groups[2])
        gcomb("s7", "u7", 3, "c", -3, 3, 3, g.groups[2])
    if E.oo_rows is not None:
        comb("a2o", "oo", 1, "r")
        comb("s2o", "a2o", 1, "c")
        comb("a4o", "s2o", 2, "r")
        comb("s4o", "a4o", 2, "c")
        if g.ring[3] is not None:
            comb("a8o", "s4o", 4, "r")
            comb("s8o", "a8o", 4, "c")
            gcomb("w10", "s8o", 2, "r", -5, 1, 3, g.groups[3])
            gcomb("s10", "w10", 2, "c", -5, 1, 1, g.groups[3])

    # ---- blend ----
    if any(g.ring[r] is not None for r in (3, 2, 1, 0)):
        be.dma_strip(it)
    for r in (3, 2, 1, 0):
        if g.ring[r] is None:
            continue
        gg = g.ring[r]
        sname, roff, coff = BLEND_SRC[r]
        st, sr0, sc0 = P[sname]

        def seg(rlo, rhi, clo, chi, key, masked, on_act=False):
            if chi <= clo:
                return
            ra, rb = rlo + roff - sr0, rhi + roff - sr0
            ca, cb = clo + coff - sc0, chi + coff - sc0
            assert ra >= 0 and ca >= 0, (it, r)
            data = st[:, ra:rb, ca:cb]
            dst = out16[:, rlo - y0:rhi - y0, clo:chi]
            if masked:
                be.cp(dst, be.strip_ap(it, key), data)
            elif on_act:
                be.acopy(dst, data)
            else:
                be.scopy(dst, data)

        if r in (1, 2, 3):
            for gi, (rlo, rhi, clo, chi) in enumerate(g.groups[r]):
                seg(rlo, rhi, clo, chi, (r, gi), True)
        elif r == 0 and g.sq0 is not None:
            ja, jb = g.sq0
            seg(gg["rlo"], gg["rhi"], gg["clo"], ja, "0L", True)
            seg(gg["rlo"], gg["rhi"], ja, jb, None, False, on_act=True)
            seg(gg["rlo"], gg["rhi"], jb, gg["chi"], "0R", True)
        else:
            seg(gg["rlo"], gg["rhi"], gg["clo"], gg["chi"], r, True)

    be.dma_out(it, out16)


def _emit_program(be):
    for it in range(NBANDS):
        _emit_band(be, it)


def prep_input(x1):
    """x1 [C, 448, 448] fp32 -> [2, C, 448, 250] fp16 parity-split blob.
    Per (wg, c, row): [zE (125) | pad | zO (124)], wg1 mirrored, NEG pads.
    Pure layout marshalling (cast/reorder/pad) -- no arithmetic."""
    xz = np.full((2, C, IN, 250), NEG, np.float16)
    xz[0, :, :, 6:125] = x1[:, :, 0:237:2]       # zE0[e]=x[2e-12]
    xz[0, :, :, 132:250] = x1[:, :, 1:236:2]     # zO0[e]=x[2e-11]
    xz[1, :, :, 6:125] = x1[:, :, 447:209:-2]    # zE1[e]=x[459-2e]
    xz[1, :, :, 132:250] = x1[:, :, 446:210:-2]  # zO1[e]=x[458-2e]
    return xz


# ---------------- numpy backend (validation) ----------------


class NumpyBE:
    def __init__(self, x):
        self.xz = prep_input(x).astype(np.float32)
        f32 = np.float32
        self._azeo = np.full((128, 64, 250), np.nan, f32)
        self.Ew = np.full((128, 64, 124), np.nan, f32)
        self.Ow = np.full((128, 64, 124), np.nan, f32)
        self.owlast = np.full((128, 1, 124), np.nan, f32)
        self.ee = np.full((128, 46, 124), np.nan, f32)
        self.oo = np.full((128, 46, 124), np.nan, f32)
        self.y = np.full((C, OUT, OUT), np.nan, f32)
        self._chunk = None
        self._out = None
        self._flip = None

    def azeo(self):
        return self._azeo

    def lv(self, name):
        nr, nc = TAG_MAX[LV_TAG[name]]
        return np.full((128, nr, nc), np.nan, np.float32)

    def out16(self):
        self._out = np.full((128, 32, OW), np.nan, np.float32)
        return self._out

    def memset(self, ap, v):
        ap[...] = v

    def max2(self, d, a, b):
        assert d.shape == a.shape == b.shape, (d.shape, a.shape, b.shape)
        np.maximum(a, b, out=d)

    def scopy(self, d, s):
        d[...] = s

    acopy = scopy
    gcopy = scopy
    cast = scopy

    def cp(self, out, mask, data):
        assert out.shape == mask.shape == data.shape
        out[...] = np.where(mask != 0, data, out)

    def dma_in(self, it, c, az):
        r0 = 64 * it + 32 * c
        az[0:64, 32 * c:32 * c + 32, :] = self.xz[0, :, r0:r0 + 32, :]
        az[64:128, 32 * c:32 * c + 32, :] = self.xz[1, :, r0:r0 + 32, :]

    def dma_rows(self, az, a, b):
        az[0:64, a:b, :] = self.xz[0, :, a:b, :]
        az[64:128, a:b, :] = self.xz[1, :, a:b, :]

    def dma_strip(self, it):
        pass

    def strip_ap(self, it, r):
        start, _ = STRIP_BANDS[it]
        off, nr, nc = STRIP_OFFS[it][r]
        return STRIP_BLOB[:, start + off:start + off + nr * nc].reshape(
            128, nr, nc)

    def dma_out(self, it, out16):
        g = GEOMS[it]
        self.y[:, g.y0:g.y1, 0:OW] = out16[0:64, 0:g.H, :]
        self.y[:, g.y0:g.y1, OW:OUT] = out16[64:128, 0:g.H, ::-1]


def numpy_kernel(x1):
    """x1: [64, 448, 448] -> [64, 224, 224] (fp32, exact clip semantics)."""
    be = NumpyBE(x1)
    _emit_program(be)
    assert not np.isnan(be.y).any(), "uncovered output pixels"
    return be.y


# ---------------- bass backend ----------------


def split_multi_waits(nc):
    """walrus CoreV3Gen accepts at most 1 sync-wait per instruction; Tile's
    tail drains can carry 2+.  Peel extras onto preceding NoOps."""
    n = 0
    for fn in nc.m.functions:
        for bb in fn.blocks:
            insts = list(bb.instructions)
            out = []
            for ins in insts:
                si = getattr(ins, "sync_info", None)
                if si is not None and len(si.on_wait) > 1:
                    waits = list(si.on_wait)
                    for k, w in enumerate(waits[:-1]):
                        nop = mybir.InstNoOp(
                            name=f"{ins.name}-waitsplit{k}",
                            engine=ins.engine, ins=[], outs=[])
                        nop.sync_info = mybir.SyncInfo(
                            on_wait=[w], on_update=[])
                        out.append(nop)
                        n += 1
                    ins.sync_info = mybir.SyncInfo(
                        on_wait=[waits[-1]], on_update=list(si.on_update))
                out.append(ins)
            if n:
                bb.instructions = out
    return n


class BassBE:
    def __init__(self, nc, pools, x, y, strips):
        self.nc = nc
        self.x = x
        self.y = y
        self.strips = strips
        pers, self.lvpool, self.iop, self.chpool = pools
        f32 = mybir.dt.float32
        self.Ew = pers.tile([128, 64, 124], DT, tag="Ew")
        self.Ow = pers.tile([128, 64, 124], DT, tag="Ow")
        self.owlast = pers.tile([128, 1, 124], DT, tag="owlast")
        self.ee = pers.tile([128, 46, 124], DT, tag="ee")
        self.oo = pers.tile([128, 46, 124], DT, tag="oo")
        self._f32 = f32
        self._strip = None

    def azeo(self):
        return self.chpool.tile([128, 64, 250], DT, tag="azeo", name="azeo")

    def lv(self, name):
        nr, nc_ = TAG_MAX[LV_TAG[name]]
        return self.lvpool.tile([128, nr, nc_], DT, tag=LV_TAG[name], name=f"lv_{name}")

    def out16(self):
        return self.iop.tile([128, 32, OW], DT, tag="out16", name="out16")

    def memset(self, ap, v):
        self.nc.gpsimd.memset(ap, v)

    def max2(self, d, a, b):
        self.nc.vector.tensor_tensor(d, a, b, MX)

    def scopy(self, d, s):
        self.nc.vector.tensor_scalar_max(d, s, NEG)

    def acopy(self, d, s):
        self.nc.scalar.copy(d, s)

    gcopy = acopy

    def cast(self, d, s):
        self.nc.scalar.copy(d, s)

    def cp(self, out, mask, data):
        self.nc.vector.copy_predicated(out, mask, data)

    def dma_in(self, it, c, az):
        r0 = 64 * it + 32 * c
        self.nc.sync.dma_start(az[0:64, 32 * c:32 * c + 32, :],
                               self.x[0, :, r0:r0 + 32, :])
        self.nc.sync.dma_start(az[64:128, 32 * c:32 * c + 32, :],
                               self.x[1, :, r0:r0 + 32, :])

    def dma_rows(self, az, a, b):
        self.nc.sync.dma_start(az[0:64, a:b, :], self.x[0, :, a:b, :])
        self.nc.sync.dma_start(az[64:128, a:b, :], self.x[1, :, a:b, :])

    def dma_strip(self, it):
        start, sz = STRIP_BANDS[it]
        self._strip = self.iop.tile([128, STRIP_MAX], mybir.dt.uint8,
                                    tag="strip", name="strip")
        self.nc.sync.dma_start(self._strip[:, 0:sz],
                               self.strips[:, start:start + sz])

    def strip_ap(self, it, r):
        off, nr, nc_ = STRIP_OFFS[it][r]
        return self._strip[:, off:off + nr * nc_].rearrange(
            "p (r c) -> p r c", c=nc_)

    def dma_out(self, it, out16):
        g = GEOMS[it]
        self.nc.sync.dma_start(self.y[0, :, g.y0:g.y1, :],
                               out16[0:64, 0:g.H, :])
        self.nc.sync.dma_start(self.y[1, :, g.y0:g.y1, :],
                               out16[64:128, 0:g.H, :])


def _emit_kernel(nc: bass.Bass):
    x = nc.dram_tensor("x", [2, C, IN, 250], DT,
                       kind="ExternalInput")
    y = nc.dram_tensor("y", [2, C, OUT, OW], DT,
                       kind="ExternalOutput")
    strips = nc.inline_tensor(STRIP_BLOB, name="mstrips")

    with TileContext(nc) as tc:
        with tc.tile_pool(name="pp", bufs=1) as pers, \
             tc.tile_pool(name="lv", bufs=1) as lvpool, \
             tc.tile_pool(name="io", bufs=1) as iop, \
             tc.tile_pool(name="ch", bufs=2) as chpool:
            be = BassBE(nc, (pers, lvpool, iop, chpool), x, y, strips)
            _emit_program(be)
    return nc


_CACHED = {}


def _get_nc():
    if "nc" not in _CACHED:
        nc = bass.Bass()
        _emit_kernel(nc)
        split_multi_waits(nc)
        _CACHED["nc"] = nc
    return _CACHED["nc"]


def kernel(x: np.ndarray) -> np.ndarray:
    nc = _get_nc()
    in_maps = [{"x": prep_input(x[b].astype(np.float32))}
               for b in range(B)]
    res = run_bass_kernel_spmd(nc, in_maps, core_ids=list(range(B)))
    out = np.empty((B, C, OUT, OUT), np.float32)
    for b, r in enumerate(res.results):
        yw = r["y"].astype(np.float32)      # [2, C, 224, 112]
        out[b, :, :, 0:OW] = yw[0]
        out[b, :, :, OW:OUT] = yw[1][:, :, ::-1]
    return out



# revision 5
# speedup vs baseline: 1.0476x; 1.0476x over previous
"""Trainium2 Bass kernel for CenterDependentPool2D (v4).

Input  x: (8, 64, 448, 448) fp32  ->  Output: (8, 64, 224, 224) fp32.

Per core = one batch element.  Partition p = c + 64*wg: channel c, wg 0 =
out cols 0..111 (natural j), wg 1 = out cols 223..112 (MIRRORED local j).
Host prep emits a parity-split fp16 blob; device computes E/O pair arrays
and shifted-max doubling pyramids for the 5 ring windows (k in
{2,8,14,20,26}), blending by ring masks.

v4 over v3: per-band EXACT needed masks are backward-propagated through
the pyramid DAG and each level is emitted as a DP-chosen set of row-group
rectangles (tight col bounds, optional gap split) instead of a single
bounding-box hull; blend rectangles get unconditional-interior splits
(interior -> Activation-engine copy, boundary strips -> masked
copy_predicated on DVE); carries are column-gated; DMA uses flattened
[128, ...] tensors for bigger descriptors.
"""

import numpy as np

import concourse.bass as bass
import concourse.mybir as mybir
from concourse.tile import TileContext
from concourse.bass_utils import run_bass_kernel_spmd

# ---------------- problem constants ----------------
B, C, IN, OUT = 8, 64, 448, 224
CEN = 112
OW = 112
NEG = -30000.0
RADII = (60, 75, 90, 105)
DT = mybir.dt.float16
MX = mybir.AluOpType.max

# out-row bands: [0,24), [24,56), ..., [184,216), [216,224)
BANDS = [(0, 24)] + [(24 + 32 * k, 56 + 32 * k) for k in range(6)] \
    + [(216, 224)]
NBANDS = len(BANDS)

CANV_R, CANV_C = 46, 132          # per-band level canvas (abs rows OB..OB+46)
EOW = 125                         # Ew/Ow/ee/oo tile width (cols used <= 124)

# DP cost constants (ns)
TT_ELEM = 0.52
CP_ELEM = 0.90
OP_OH = 180.0
ACT_ELEM = 0.83
ACT_OH = 280.0

# ---------------- static geometry ----------------

_yy, _xx = np.mgrid[0:OUT, 0:OUT]
_D2 = (_yy - CEN) ** 2 + (_xx - CEN) ** 2
NESTED = np.stack([(_D2 < R * R) for R in RADII])
RING_ID = 4 - NESTED.sum(0)


def _localize(a):
    return a[:, 0:CEN], a[:, ::-1][:, 0:CEN]


_R0, _R1 = _localize(RING_ID)
# union/both ring-cell masks in localized coords
RING_ANY = [np.asarray((_R0 == r) | (_R1 == r)) for r in range(5)]
RING_BOTH = [np.asarray((_R0 == r) & (_R1 == r)) for r in range(5)]

# blend source per ring: (level, row_off, col_off): out (R, J) reads
# level[R + roff, J + coff]
BLEND_SRC = {4: ("s13", -6, 0), 3: ("s10", -5, 1), 2: ("s7", -3, 3),
             1: ("s4o", -2, 4), 0: ("ee", 0, 6)}

# pyramid DAG: level -> (src, shift, axis)
LEVELS = {
    "a2": ("ee", 1, "r"), "s2": ("a2", 1, "c"),
    "a4": ("s2", 2, "r"), "s4": ("a4", 2, "c"),
    "a8": ("s4", 4, "r"), "s8": ("a8", 4, "c"),
    "v13": ("s8", 5, "r"), "s13": ("v13", 5, "c"),
    "u7": ("s4", 3, "r"), "s7": ("u7", 3, "c"),
    "a2o": ("oo", 1, "r"), "s2o": ("a2o", 1, "c"),
    "a4o": ("s2o", 2, "r"), "s4o": ("a4o", 2, "c"),
    "a8o": ("s4o", 4, "r"), "s8o": ("a8o", 4, "c"),
    "w10": ("s8o", 2, "r"), "s10": ("w10", 2, "c"),
}
# realization order: consumers before producers
REV_ORDER = ["s13", "v13", "s8", "s7", "a8", "u7", "s4", "a4", "s2", "a2",
             "s10", "w10", "s8o", "a8o", "s4o", "a4o", "s2o", "a2o"]


def decompose(mask, elem_ns=TT_ELEM, oh=OP_OH):
    """mask: bool [R, C] -> list of rects (r0, r1, c0, c1) covering mask.
    DP over row boundaries; per group tight col bbox, optional split into
    2 col intervals at the largest internal gap."""
    R, Cc = mask.shape
    rows_any = mask.any(1)
    rects = []
    r = 0
    while r < R:
        if not rows_any[r]:
            r += 1
            continue
        e = r
        while e < R and rows_any[e]:
            e += 1
        rects.extend(_dp_run(mask, r, e, elem_ns, oh))
        r = e
    return rects


def _group_cost_and_rects(mask, a, b, elem_ns, oh):
    sub = mask[a:b]
    cols = sub.any(0)
    ci = np.where(cols)[0]
    clo, chi = int(ci.min()), int(ci.max()) + 1
    nr = b - a
    best = (nr * (chi - clo) * elem_ns + oh, [(a, b, clo, chi)])
    # largest internal gap
    gaps = np.where(~cols[clo:chi])[0]
    if len(gaps):
        # find longest run of gaps
        runs = np.split(gaps, np.where(np.diff(gaps) != 1)[0] + 1)
        run = max(runs, key=len)
        g0, g1 = clo + int(run[0]), clo + int(run[-1]) + 1
        c2 = (nr * ((chi - clo) - (g1 - g0))) * elem_ns + 2 * oh
        if c2 < best[0]:
            best = (c2, [(a, b, clo, g0), (a, b, g1, chi)])
    return best


def _dp_run(mask, r0, r1, elem_ns, oh):
    n = r1 - r0
    INF = float("inf")
    dp = [INF] * (n + 1)
    choice = [None] * (n + 1)
    dp[0] = 0.0
    for b in range(1, n + 1):
        for a in range(max(0, b - 48), b):
            c, rects = _group_cost_and_rects(mask, r0 + a, r0 + b,
                                             elem_ns, oh)
            if dp[a] + c < dp[b]:
                dp[b] = dp[a] + c
                choice[b] = (a, rects)
    out = []
    b = n
    while b > 0:
        a, rects = choice[b]
        out.extend(rects)
        b = a
    out.reverse()
    return out


def _paint(canvas, rects):
    for (a, b, c, d) in rects:
        canvas[a:b, c:d] = True


def _shift_req(req_canvas, rects, d, axis):
    """src required at rect and rect shifted +d along axis."""
    for (a, b, c, e) in rects:
        req_canvas[a:b, c:e] = True
        if axis == "r":
            req_canvas[a + d:b + d, c:e] = True
        else:
            req_canvas[a:b, c + d:e + d] = True


class BandPlan:
    """Per-band exact-mask plan: blend segments, level groups, base-array
    groups, carry col intervals."""

    def __init__(self, it, carry_ee_cols, carry_oo_cols, next_oo13_cols):
        self.it = it
        y0, y1 = BANDS[it]
        self.y0, self.y1, self.H = y0, y1, y1 - y0
        OB = 32 * it - 14
        self.OB = OB
        req = {n: np.zeros((CANV_R, CANV_C), bool) for n in LEVELS}
        req["ee"] = np.zeros((CANV_R, CANV_C), bool)
        req["oo"] = np.zeros((CANV_R, CANV_C), bool)

        # ---- blends ----
        # ring masks within band rows, in (canvas-row-of-out-row, col):
        # out row R -> blend writes; source level coords = (R+roff, J+coff).
        rows = slice(y0, y1)
        self.blend = {}          # ring -> list of segments
        # segment: (kind, rlo, rhi, clo, chi) kind in {"cp", "act"}
        for r in (4, 3, 2, 1, 0):
            any_m = RING_ANY[r][rows]
            if not any_m.any():
                self.blend[r] = []
                continue
            both_m = RING_BOTH[r][rows]
            segs = []
            if r == 4:
                # unmasked write; don't-care anywhere (later cps fix rest)
                rects = decompose(any_m, TT_ELEM, OP_OH)
                for (a, b, c, d) in rects:
                    segs.append(("s13w", y0 + a, y0 + b, c, d))
            else:
                rects = decompose(any_m, CP_ELEM, OP_OH)
                for (a, b, c, d) in rects:
                    # unconditional interior: cols where all rows in group
                    # are true in BOTH wg masks
                    sub = both_m[a:b, c:d]
                    allin = sub.all(0)
                    ji = np.where(allin)[0]
                    ja = jb = None
                    if len(ji):
                        # largest contiguous all-true run
                        runs = np.split(ji, np.where(np.diff(ji) != 1)[0] + 1)
                        run = max(runs, key=len)
                        if len(run) * (b - a) >= 400:
                            ja, jb = c + int(run[0]), c + int(run[-1]) + 1
                    if ja is None:
                        segs.append(("cp", y0 + a, y0 + b, c, d))
                    else:
                        if ja > c:
                            segs.append(("cp", y0 + a, y0 + b, c, ja))
                        segs.append(("act", y0 + a, y0 + b, ja, jb))
                        if d > jb:
                            segs.append(("cp", y0 + a, y0 + b, jb, d))
            self.blend[r] = segs
            # source requirements (full rects incl. masked cells)
            lvl, roff, coff = BLEND_SRC[r]
            for (_k, rlo, rhi, clo, chi) in segs:
                ra, rb = rlo + roff - OB, rhi + roff - OB
                ca, cb = clo + coff, chi + coff
                assert 0 <= ra and rb <= CANV_R and cb <= CANV_C, (it, r)
                req[lvl][ra:rb, ca:cb] = True

        # ---- levels (reverse topo) ----
        self.groups = {}         # level -> list of canvas rects
        for name in REV_ORDER:
            m = req[name]
            if not m.any():
                self.groups[name] = []
                continue
            rects = decompose(m, TT_ELEM, OP_OH)
            self.groups[name] = rects
            src, d, axis = LEVELS[name]
            _shift_req(req[src], rects, d, axis)

        # ---- ee / oo ----
        # carry-in requirement from next band (rows 32:46 here = next 0:14)
        if carry_ee_cols is not None:
            req["ee"][32:46] |= carry_ee_cols
        if carry_oo_cols is not None:
            req["oo"][32:46] |= carry_oo_cols
        self.req_ee = req["ee"]
        self.req_oo = req["oo"]
        # carry-out requirement to previous band (oo row 13 excluded: it is
        # always rewritten by the oo13 special op, carry content don't-care)
        self.carry_ee = req["ee"][0:14].copy()
        self.carry_oo = req["oo"][0:14].copy()
        self.carry_oo[13] = False
        # oo row 13 special cols
        oi = np.where(req["oo"][13])[0]
        self.oo13 = (int(oi.min()), int(oi.max()) + 1) if len(oi) else None
        # fresh realizations
        ee_fresh = np.zeros_like(req["ee"])
        ee_fresh[14:46] = req["ee"][14:46]
        if it == 7:
            self.ee_groups = []          # memset instead
            self.oo_groups = []
        else:
            force = [22, 30] if it == 0 else [30]
            self.ee_groups = self._split_rows(ee_fresh, force)
            oo_fresh = np.zeros_like(req["oo"])
            oo_fresh[14:45] = req["oo"][14:45]
            self.oo_groups = self._split_rows(oo_fresh, [29])
        # carry copy col intervals (<=2) for rows 0:14
        self.carry_ee_copy = self._carry_cols(self.carry_ee)
        self.carry_oo_copy = self._carry_cols(self.carry_oo)

        # ---- Ew / Ow ----
        # Ew row 2j, 2j+1 needed at ee fresh row 14+j cols; canvas [64, C]
        if it == 7:
            self.ew_groups = []
            self.ow_groups = []
            self.owlast_cols = None
        else:
            ewm = np.zeros((64, CANV_C), bool)
            for (a, b, c, d) in self.ee_groups:
                j0, j1 = a - 14, b - 14
                ewm[2 * j0:2 * j1, c:d] = True
            force = [16, 32] if it == 0 else [32]
            self.ew_groups = self._split_rows_generic(ewm, force)
            owm = np.zeros((64, CANV_C), bool)
            for (a, b, c, d) in self.oo_groups:
                j0, j1 = a - 14, b - 14
                owm[2 * j0 + 1:2 * j1 + 1, c:d] = True
            if self.oo13 is not None:
                owm[0, self.oo13[0]:self.oo13[1]] = True
            if next_oo13_cols is not None:
                owm[63, next_oo13_cols[0]:next_oo13_cols[1]] = True
                self.owlast_cols = next_oo13_cols
            else:
                self.owlast_cols = None
            self.ow_groups = self._split_rows_generic(owm, force)

    @staticmethod
    def _split_rows(mask, boundaries):
        rects = []
        bounds = [0] + boundaries + [CANV_R]
        for a, b in zip(bounds, bounds[1:]):
            sub = np.zeros_like(mask)
            sub[a:b] = mask[a:b]
            rects.extend(decompose(sub, TT_ELEM, OP_OH))
        return rects

    @staticmethod
    def _split_rows_generic(mask, boundaries):
        R = mask.shape[0]
        rects = []
        bounds = [0] + [b for b in boundaries if 0 < b < R] + [R]
        for a, b in zip(bounds, bounds[1:]):
            sub = np.zeros_like(mask)
            sub[a:b] = mask[a:b]
            rects.extend(decompose(sub, TT_ELEM, OP_OH))
        return rects

    @staticmethod
    def _carry_cols(mask):
        """rows 0:14 carry mask -> list of (c0, c1) intervals (<=2)."""
        cols = mask.any(0)
        ci = np.where(cols)[0]
        if not len(ci):
            return []
        clo, chi = int(ci.min()), int(ci.max()) + 1
        gaps = np.where(~cols[clo:chi])[0]
        if len(gaps) >= 16:
            runs = np.split(gaps, np.where(np.diff(gaps) != 1)[0] + 1)
            run = max(runs, key=len)
            if len(run) >= 16:
                g0, g1 = clo + int(run[0]), clo + int(run[-1]) + 1
                return [(clo, g0), (g1, chi)]
        return [(clo, chi)]


def _build_plans():
    plans = [None] * NBANDS
    carry_ee = carry_oo = None
    next_oo13 = None
    for it in range(NBANDS - 1, -1, -1):
        p = BandPlan(it, carry_ee, carry_oo, next_oo13)
        plans[it] = p
        carry_ee, carry_oo = p.carry_ee, p.carry_oo
        next_oo13 = p.oo13
    return plans


PLANS = _build_plans()

# ---- validate coverage: realized(src) must cover all reads ----


def _validate_plans():
    for it, p in enumerate(PLANS):
        real = {}
        for name in list(LEVELS) + ["ee", "oo"]:
            cv = np.zeros((CANV_R, CANV_C), bool)
            if name == "ee":
                _paint(cv, p.ee_groups)
                cv[0:14] = True if it > 0 else False
                if it == 0:
                    cv[0:14] = True     # memset
                if it == 7:
                    cv[14:46] = True    # memset
                # carry rows realized iff prev band realized 32:46 there —
                # checked via carry_ee ⊆ prev realized below
            elif name == "oo":
                _paint(cv, p.oo_groups)
                cv[0:14] = True
                if p.oo13 is not None:
                    cv[13, p.oo13[0]:p.oo13[1]] = True
                if it == 7:
                    cv[14:46] = True
            else:
                _paint(cv, p.groups.get(name, []))
            real[name] = cv
        # each level's reads covered by src realization
        for name in REV_ORDER:
            rects = p.groups.get(name, [])
            if not rects:
                continue
            src, d, axis = LEVELS[name]
            need = np.zeros((CANV_R, CANV_C), bool)
            _shift_req(need, rects, d, axis)
            assert not (need & ~real[src]).any(), (it, name, src)
        # blend reads covered
        for r, segs in p.blend.items():
            lvl, roff, coff = BLEND_SRC[r]
            for (_k, rlo, rhi, clo, chi) in segs:
                ra, rb = rlo + roff - p.OB, rhi + roff - p.OB
                sub = real[lvl][ra:rb, clo + coff:chi + coff]
                assert sub.all(), (it, r, _k)
        # carry feasibility: this band's carry req ⊆ prev band's realized
        if it > 0:
            prev = PLANS[it - 1]
            pr = np.zeros((CANV_R, CANV_C), bool)
            _paint(pr, prev.ee_groups)
            if it - 1 == 0:
                pr[0:14] = True
            assert not (p.carry_ee & ~pr[32:46]).any(), (it, "carry_ee")
            po = np.zeros((CANV_R, CANV_C), bool)
            _paint(po, prev.oo_groups)
            if prev.oo13 is not None:
                po[13, prev.oo13[0]:prev.oo13[1]] = True
            assert not (p.carry_oo & ~po[32:46]).any(), (it, "carry_oo")


_validate_plans()

# ---- tile sizing ----

LV_TAG = dict(a2="tP", a4="tP", a8="tP", v13="tP",
              s2="tQ", s8="tQ", s4="tS4", u7="tT", w10="tT",
              s7="tS7", a2o="tPo", a4o="tPo", a8o="tPo",
              s2o="tQo", s8o="tQo", s4o="tS4o", s10="tS10")


def _bbox(rects):
    r0 = min(a for a, b, c, d in rects)
    r1 = max(b for a, b, c, d in rects)
    c0 = min(c for a, b, c, d in rects)
    c1 = max(d for a, b, c, d in rects)
    return r0, r1, c0, c1


LV_ORIGIN = []        # per band: level -> (r0, c0) canvas origin of tile
TAG_MAX = {}
for _p in PLANS:
    org = {}
    for _n in LEVELS:
        rects = _p.groups.get(_n, [])
        if not rects:
            continue
        r0, r1, c0, c1 = _bbox(rects)
        org[_n] = (r0, c0)
        if _n == "s13":
            continue
        t = LV_TAG[_n]
        sz = TAG_MAX.get(t, (0, 0))
        TAG_MAX[t] = (max(sz[0], r1 - r0), max(sz[1], c1 - c0))
    LV_ORIGIN.append(org)

# ---- blend mask strips ----


def _strip_mask(it, rlo, rhi, clo, chi, ring):
    y0 = PLANS[it].y0
    n0 = (_R0 == ring)[rlo:rhi, clo:chi].astype(np.uint8)
    n1 = (_R1[:, 0:CEN] if False else (_R1 == ring))[rlo:rhi, clo:chi] \
        .astype(np.uint8)
    m = np.zeros((128, rhi - rlo, chi - clo), np.uint8)
    m[0:64] = n0[None]
    m[64:128] = n1[None]
    return m


def _build_strips():
    blobs, bands, offs = [], [], []
    pos = 0
    for it, p in enumerate(PLANS):
        start = pos
        ent = {}
        for r in (3, 2, 1, 0):
            for gi, seg in enumerate(p.blend.get(r, [])):
                kind, rlo, rhi, clo, chi = seg
                if kind != "cp":
                    continue
                m = _strip_mask(it, rlo, rhi, clo, chi, r)
                nr, nc = m.shape[1], m.shape[2]
                ent[(r, gi)] = (pos - start, nr, nc)
                blobs.append(np.ascontiguousarray(m).reshape(128, -1))
                pos += nr * nc
        offs.append(ent)
        bands.append((start, pos - start))
    blob = (np.concatenate(blobs, 1) if blobs
            else np.zeros((128, 1), np.uint8))
    return blob, bands, offs


STRIP_BLOB, STRIP_BANDS, STRIP_OFFS = _build_strips()
STRIP_MAX = max(max(sz for _, sz in STRIP_BANDS), 1)

# ---------------- shared band program ----------------


def _emit_band(be, it):
    p = PLANS[it]
    y0, y1, H, OB = p.y0, p.y1, p.H, p.OB
    ee, oo, Ew, Ow, ol = be.ee, be.oo, be.Ew, be.Ow, be.owlast

    # ---- input DMA ----
    if 0 < it < 7:
        az = be.azeo()
        be.dma_band(it, az)
    elif it == 0:
        az = be.azeo()
        be.dma_rows(az, 0, 16)
        be.dma_rows(az, 16, 32)
        be.dma_rows(az, 32, 64)

    # ---- Ew / Ow builds ----
    if it < 7:
        for (a, b, c, d) in p.ew_groups:
            be.max2(Ew[:, a:b, c:d], az[:, a:b, c:d], az[:, a:b, 126 + c:126 + d])
        for (a, b, c, d) in p.ow_groups:
            be.max2(Ow[:, a:b, c:d], az[:, a:b, 126 + c:126 + d],
                    az[:, a:b, 1 + c:1 + d])

    # ---- carries ----
    if it > 0:
        for (c0, c1) in p.carry_ee_copy:
            be.gcopy(ee[:, 0:14, c0:c1], ee[:, 32:46, c0:c1])
        for (c0, c1) in p.carry_oo_copy:
            be.gcopy(oo[:, 0:14, c0:c1], oo[:, 32:46, c0:c1])
    else:
        be.memset(ee[:, 0:14, :], NEG)
        be.memset(oo[:, 0:14, :], NEG)

    # ---- ee/oo fresh ----
    if it == 7:
        be.memset(ee[:, 14:46, :], NEG)
        be.memset(oo[:, 14:46, :], NEG)
        if p.oo13 is not None:
            s0, s1 = p.oo13
            be.scopy(oo[:, 13:14, s0:s1], ol[:, 0:1, s0:s1])
    else:
        for (a, b, c, d) in p.ee_groups:
            j0, j1 = a - 14, b - 14
            be.max2(ee[:, a:b, c:d], Ew[:, 2 * j0:2 * j1:2, c:d],
                    Ew[:, 2 * j0 + 1:2 * j1:2, c:d])
        if p.oo13 is not None:
            s0, s1 = p.oo13
            if it == 0:
                be.scopy(oo[:, 13:14, s0:s1], Ow[:, 0:1, s0:s1])
            else:
                be.max2(oo[:, 13:14, s0:s1], ol[:, 0:1, s0:s1],
                        Ow[:, 0:1, s0:s1])
        for (a, b, c, d) in p.oo_groups:
            j0, j1 = a - 14, b - 14
            be.max2(oo[:, a:b, c:d], Ow[:, 2 * j0 + 1:2 * j1 + 1:2, c:d],
                    Ow[:, 2 * j0 + 2:2 * j1 + 2:2, c:d])
        if p.owlast_cols is not None:
            g0, g1 = p.owlast_cols
            be.acopy(ol[:, 0:1, g0:g1], Ow[:, 63:64, g0:g1])

    # ---- pyramid ----
    out16 = be.out16()
    tiles = {"ee": (ee, 0, 0), "oo": (oo, 0, 0)}
    org = LV_ORIGIN[it]

    def src_ap(st, sr0, sc0, a, b, c, d):
        return st[:, a - sr0:b - sr0, c - sc0:d - sc0]

    emit_order = ["a2", "a2o", "s2", "s2o", "a4", "a4o", "s4", "s4o",
                  "a8", "a8o", "u7", "s8", "s8o", "s7", "w10", "v13",
                  "s10", "s13"]
    for name in emit_order:
        rects = p.groups.get(name, [])
        if not rects:
            continue
        srcn, dsh, axis = LEVELS[name]
        st, sr0, sc0 = tiles[srcn]
        if name == "s13":
            # write directly into out16: out row = canvas row + OB + 6
            for (a, b, c, d) in rects:
                ra, rb = a + OB + 6 - y0, b + OB + 6 - y0
                dst = out16[:, ra:rb, c:d]
                be.max2(dst, src_ap(st, sr0, sc0, a, b, c, d),
                        src_ap(st, sr0, sc0, a, b, c + dsh, d + dsh))
            continue
        r0, c0 = org[name]
        t = be.lv(name)
        tiles[name] = (t, r0, c0)
        for (a, b, c, d) in rects:
            dst = t[:, a - r0:b - r0, c - c0:d - c0]
            if axis == "r":
                be.max2(dst, src_ap(st, sr0, sc0, a, b, c, d),
                        src_ap(st, sr0, sc0, a + dsh, b + dsh, c, d))
            else:
                be.max2(dst, src_ap(st, sr0, sc0, a, b, c, d),
                        src_ap(st, sr0, sc0, a, b, c + dsh, d + dsh))

    # ---- blends (rings 3,2,1,0 after s13 write) ----
    if any(k[0] in (0, 1, 2, 3) for k in STRIP_OFFS[it]):
        be.dma_strip(it)
    for r in (3, 2, 1, 0):
        lvl, roff, coff = BLEND_SRC[r]
        st, sr0, sc0 = tiles.get(lvl, (None, 0, 0))
        for gi, seg in enumerate(p.blend.get(r, [])):
            kind, rlo, rhi, clo, chi = seg
            ra, rb = rlo + roff - OB, rhi + roff - OB
            data = st[:, ra - sr0:rb - sr0,
                      clo + coff - sc0:chi + coff - sc0]
            dst = out16[:, rlo - y0:rhi - y0, clo:chi]
            if kind == "cp":
                be.cp(dst, be.strip_ap(it, (r, gi)), data)
            else:
                be.acopy(dst, data)

    be.dma_out(it, out16)


def _emit_program(be):
    for it in range(NBANDS):
        _emit_band(be, it)


def prep_input(x1):
    """x1 [C, 448, 448] fp32 -> [128, 448, 250] fp16 parity-split blob.
    Per (wg*64+c, row): [pad6 | zE (119) | pad7 | zO (118)], wg1 mirrored,
    NEG pads.  Pure layout marshalling (cast/reorder/pad), no arithmetic."""
    xz = np.full((2, C, IN, 250), NEG, np.float16)
    xz[0, :, :, 6:125] = x1[:, :, 0:237:2]       # zE0[e]=x[2e-12]
    xz[0, :, :, 132:250] = x1[:, :, 1:236:2]     # zO0[e]=x[2e-11]
    xz[1, :, :, 6:125] = x1[:, :, 447:209:-2]    # zE1[e]=x[459-2e]
    xz[1, :, :, 132:250] = x1[:, :, 446:210:-2]  # zO1[e]=x[458-2e]
    return np.ascontiguousarray(xz.reshape(128, IN, 250))


# ---------------- numpy backend (validation) ----------------


class NumpyBE:
    def __init__(self, x):
        self.xz = prep_input(x).astype(np.float32)
        f32 = np.float32
        self._azeo = np.full((128, 64, 250), np.nan, f32)
        self.Ew = np.full((128, 64, EOW), np.nan, f32)
        self.Ow = np.full((128, 64, EOW), np.nan, f32)
        self.owlast = np.full((128, 1, EOW), np.nan, f32)
        self.ee = np.full((128, 46, EOW), np.nan, f32)
        self.oo = np.full((128, 46, EOW), np.nan, f32)
        self.y = np.full((128, OUT, OW), np.nan, f32)
        self._out = None

    def azeo(self):
        return self._azeo

    def lv(self, name):
        nr, nc = TAG_MAX[LV_TAG[name]]
        return np.full((128, nr, nc), np.nan, np.float32)

    def out16(self):
        self._out = np.full((128, 32, OW), np.nan, np.float32)
        return self._out

    def memset(self, ap, v):
        ap[...] = v

    def max2(self, d, a, b):
        assert d.shape == a.shape == b.shape, (d.shape, a.shape, b.shape)
        np.maximum(a, b, out=d)

    def scopy(self, d, s):
        d[...] = s

    acopy = scopy
    gcopy = scopy

    def cp(self, out, mask, data):
        assert out.shape == mask.shape == data.shape
        out[...] = np.where(mask != 0, data, out)

    def dma_band(self, it, az):
        r0 = 64 * it
        az[:, :, :] = self.xz[:, r0:r0 + 64, :]

    def dma_rows(self, az, a, b):
        az[:, a:b, :] = self.xz[:, a:b, :]

    def dma_strip(self, it):
        pass

    def strip_ap(self, it, key):
        start, _ = STRIP_BANDS[it]
        off, nr, nc = STRIP_OFFS[it][key]
        return STRIP_BLOB[:, start + off:start + off + nr * nc].reshape(
            128, nr, nc)

    def dma_out(self, it, out16):
        p = PLANS[it]
        self.y[:, p.y0:p.y1, :] = out16[:, 0:p.H, :]


def numpy_kernel(x1):
    """x1: [64, 448, 448] -> [64, 224, 224] (fp32, exact clip semantics)."""
    be = NumpyBE(x1)
    _emit_program(be)
    yw = be.y
    assert not np.isnan(yw).any(), "uncovered output pixels"
    out = np.empty((C, OUT, OUT), np.float32)
    out[:, :, 0:OW] = yw[0:64]
    out[:, :, OW:OUT] = yw[64:128, :, ::-1]
    return out


# ---------------- bass backend ----------------


def split_multi_waits(nc):
    """walrus CoreV3Gen accepts at most 1 sync-wait per instruction; Tile's
    tail drains can carry 2+.  Peel extras onto preceding NoOps."""
    n = 0
    for fn in nc.m.functions:
        for bb in fn.blocks:
            insts = list(bb.instructions)
            out = []
            for ins in insts:
                si = getattr(ins, "sync_info", None)
                if si is not None and len(si.on_wait) > 1:
                    waits = list(si.on_wait)
                    for k, w in enumerate(waits[:-1]):
                        nop = mybir.InstNoOp(
                            name=f"{ins.name}-waitsplit{k}",
                            engine=ins.engine, ins=[], outs=[])
                        nop.sync_info = mybir.SyncInfo(
                            on_wait=[w], on_update=[])
                        out.append(nop)
                        n += 1
                    ins.sync_info = mybir.SyncInfo(
                        on_wait=[waits[-1]], on_update=list(si.on_update))
                out.append(ins)
            if n:
                bb.instructions = out
    return n


class BassBE:
    def __init__(self, nc, pools, x, y, strips):
        self.nc = nc
        self.x = x
        self.y = y
        self.strips = strips
        pers, self.lvpool, self.iop, self.chpool, self.strippool = pools
        self.Ew = pers.tile([128, 64, EOW], DT, tag="Ew")
        self.Ow = pers.tile([128, 64, EOW], DT, tag="Ow")
        self.owlast = pers.tile([128, 1, EOW], DT, tag="owlast")
        self.ee = pers.tile([128, 46, EOW], DT, tag="ee")
        self.oo = pers.tile([128, 46, EOW], DT, tag="oo")
        self._strip = None

    def azeo(self):
        return self.chpool.tile([128, 64, 250], DT, tag="azeo", name="azeo")

    def lv(self, name):
        nr, nc_ = TAG_MAX[LV_TAG[name]]
        return self.lvpool.tile([128, nr, nc_], DT, tag=LV_TAG[name],
                                name=f"lv_{name}")

    def out16(self):
        return self.iop.tile([128, 32, OW], DT, tag="out16", name="out16")

    def memset(self, ap, v):
        self.nc.gpsimd.memset(ap, v)

    def max2(self, d, a, b):
        self.nc.vector.tensor_tensor(d, a, b, MX)

    def scopy(self, d, s):
        self.nc.vector.tensor_scalar_max(d, s, NEG)

    def acopy(self, d, s):
        self.nc.scalar.copy(d, s)

    gcopy = acopy

    def cp(self, out, mask, data):
        self.nc.vector.copy_predicated(out, mask, data)

    def dma_band(self, it, az):
        r0 = 64 * it
        self.nc.sync.dma_start(az[:, :, :], self.x[:, r0:r0 + 64, :])

    def dma_rows(self, az, a, b):
        self.nc.sync.dma_start(az[:, a:b, :], self.x[:, a:b, :])

    def dma_strip(self, it):
        start, sz = STRIP_BANDS[it]
        self._strip = self.strippool.tile([128, STRIP_MAX], mybir.dt.uint8,
                                          tag="strip", name="strip")
        self.nc.sync.dma_start(self._strip[:, 0:sz],
                               self.strips[:, start:start + sz])

    def strip_ap(self, it, key):
        off, nr, nc_ = STRIP_OFFS[it][key]
        return self._strip[:, off:off + nr * nc_].rearrange(
            "p (r c) -> p r c", c=nc_)

    def dma_out(self, it, out16):
        p = PLANS[it]
        self.nc.sync.dma_start(self.y[:, p.y0:p.y1, :],
                               out16[:, 0:p.H, :])


def _emit_kernel(nc: bass.Bass):
    x = nc.dram_tensor("x", [128, IN, 250], DT, kind="ExternalInput")
    y = nc.dram_tensor("y", [128, OUT, OW], DT, kind="ExternalOutput")
    strips = nc.inline_tensor(STRIP_BLOB, name="mstrips")

    with TileContext(nc) as tc:
        with tc.tile_pool(name="pp", bufs=1) as pers, \
             tc.tile_pool(name="lv", bufs=1) as lvpool, \
             tc.tile_pool(name="io", bufs=2) as iop, \
             tc.tile_pool(name="ch", bufs=2) as chpool, \
             tc.tile_pool(name="st", bufs=1) as strippool:
            be = BassBE(nc, (pers, lvpool, iop, chpool, strippool), x, y, strips)
            _emit_program(be)
    return nc


_CACHED = {}


def _get_nc():
    if "nc" not in _CACHED:
        nc = bass.Bass()
        _emit_kernel(nc)
        split_multi_waits(nc)
        _CACHED["nc"] = nc
    return _CACHED["nc"]


def kernel(x: np.ndarray) -> np.ndarray:
    nc = _get_nc()
    in_maps = [{"x": prep_input(x[b].astype(np.float32))}
               for b in range(B)]
    res = run_bass_kernel_spmd(nc, in_maps, core_ids=list(range(B)))
    out = np.empty((B, C, OUT, OUT), np.float32)
    for b, r in enumerate(res.results):
        yw = r["y"].astype(np.float32)      # [128, 224, 112]
        out[b, :, :, 0:OW] = yw[0:64]
        out[b, :, :, OW:OUT] = yw[64:128, :, ::-1]
    return out


# revision 22
# speedup vs baseline: 1.1506x; 1.0982x over previous
"""Trainium2 Bass kernel for CenterDependentPool2D (v4).

Input  x: (8, 64, 448, 448) fp32  ->  Output: (8, 64, 224, 224) fp32.

Per core = one batch element.  Partition p = c + 64*wg: channel c, wg 0 =
out cols 0..111 (natural j), wg 1 = out cols 223..112 (MIRRORED local j).
Host prep emits a parity-split fp16 blob; device computes E/O pair arrays
and shifted-max doubling pyramids for the 5 ring windows (k in
{2,8,14,20,26}), blending by ring masks.

v4 over v3: per-band EXACT needed masks are backward-propagated through
the pyramid DAG and each level is emitted as a DP-chosen set of row-group
rectangles (tight col bounds, optional gap split) instead of a single
bounding-box hull; blend rectangles get unconditional-interior splits
(interior -> Activation-engine copy, boundary strips -> masked
copy_predicated on DVE); carries are column-gated; DMA uses flattened
[128, ...] tensors for bigger descriptors.
"""

import numpy as np

import concourse.bass as bass
import concourse.mybir as mybir
from concourse.tile import TileContext
from concourse.bass_utils import run_bass_kernel_spmd

# ---------------- problem constants ----------------
B, C, IN, OUT = 8, 64, 448, 224
CEN = 112
OW = 112
NEG = -30000.0
RADII = (60, 75, 90, 105)
DT = mybir.dt.float16
MX = mybir.AluOpType.max

# out-row bands: [0,24), [24,56), ..., [184,216), [216,224)
BANDS = [(0, 24)] + [(24 + 32 * k, 56 + 32 * k) for k in range(6)] \
    + [(216, 224)]
NBANDS = len(BANDS)

CANV_R, CANV_C = 46, 132          # per-band level canvas (abs rows OB..OB+46)
EOW = 125                         # Ew/Ow/ee/oo tile width (cols used <= 124)

# DP cost constants (ns)
TT_ELEM = 0.52
CP_ELEM = 0.90
OP_OH = 60.0
ACT_ELEM = 0.83
ACT_OH = 280.0

# ---------------- static geometry ----------------

_yy, _xx = np.mgrid[0:OUT, 0:OUT]
_D2 = (_yy - CEN) ** 2 + (_xx - CEN) ** 2
NESTED = np.stack([(_D2 < R * R) for R in RADII])
RING_ID = 4 - NESTED.sum(0)


def _localize(a):
    return a[:, 0:CEN], a[:, ::-1][:, 0:CEN]


_R0, _R1 = _localize(RING_ID)
# union/both ring-cell masks in localized coords
RING_ANY = [np.asarray((_R0 == r) | (_R1 == r)) for r in range(5)]
RING_BOTH = [np.asarray((_R0 == r) & (_R1 == r)) for r in range(5)]

# blend source per ring: (level, row_off, col_off): out (R, J) reads
# level[R + roff, J + coff]
BLEND_SRC = {4: ("s13", -6, 0), 3: ("s10", -5, 1), 2: ("s7", -3, 3),
             1: ("s4o", -2, 4), 0: ("ee", 0, 6)}

# pyramid DAG: level -> (src, shift, axis)
LEVELS = {
    "a2": ("ee", 1, "r"), "s2": ("a2", 1, "c"),
    "a4": ("s2", 2, "r"), "s4": ("a4", 2, "c"),
    "a8": ("s4", 4, "r"), "s8": ("a8", 4, "c"),
    "v13": ("s8", 5, "r"), "s13": ("v13", 5, "c"),
    "u7": ("s4", 3, "r"), "s7": ("u7", 3, "c"),
    "a2o": ("oo", 1, "r"), "s2o": ("a2o", 1, "c"),
    "a4o": ("s2o", 2, "r"), "s4o": ("a4o", 2, "c"),
    "a8o": ("s4o", 4, "r"), "s8o": ("a8o", 4, "c"),
    "w10": ("s8o", 2, "r"), "s10": ("w10", 2, "c"),
}
# realization order: consumers before producers
REV_ORDER = ["s13", "v13", "s8", "s7", "a8", "u7", "s4", "a4", "s2", "a2",
             "s10", "w10", "s8o", "a8o", "s4o", "a4o", "s2o", "a2o"]

# persistent mid-chain levels: level -> fresh-window start lo (fresh rows
# [lo, lo+32) per band; rows [0, lo) carried from the previous band).  The
# phases fit the 46-row canvas exactly (a2/s2 pull down to rows [11,45),
# reading ee rows up to 45).
PERSIST = {"s2": 13, "s4": 11, "s8": 7, "s2o": 12, "s4o": 10, "s8o": 6}


def decompose(mask, elem_ns=TT_ELEM, oh=OP_OH):
    """mask: bool [R, C] -> list of rects (r0, r1, c0, c1) covering mask.
    DP over row boundaries; per group tight col bbox, optional split into
    2 col intervals at the largest internal gap."""
    R, Cc = mask.shape
    rows_any = mask.any(1)
    rects = []
    r = 0
    while r < R:
        if not rows_any[r]:
            r += 1
            continue
        e = r
        while e < R and rows_any[e]:
            e += 1
        rects.extend(_dp_run(mask, r, e, elem_ns, oh))
        r = e
    return rects


def _group_cost_and_rects(mask, a, b, elem_ns, oh):
    sub = mask[a:b]
    cols = sub.any(0)
    ci = np.where(cols)[0]
    clo, chi = int(ci.min()), int(ci.max()) + 1
    nr = b - a
    best = (nr * (chi - clo) * elem_ns + oh, [(a, b, clo, chi)])
    # largest internal gap
    gaps = np.where(~cols[clo:chi])[0]
    if len(gaps):
        # find longest run of gaps
        runs = np.split(gaps, np.where(np.diff(gaps) != 1)[0] + 1)
        run = max(runs, key=len)
        g0, g1 = clo + int(run[0]), clo + int(run[-1]) + 1
        c2 = (nr * ((chi - clo) - (g1 - g0))) * elem_ns + 2 * oh
        if c2 < best[0]:
            best = (c2, [(a, b, clo, g0), (a, b, g1, chi)])
    return best


def _dp_run(mask, r0, r1, elem_ns, oh):
    n = r1 - r0
    INF = float("inf")
    dp = [INF] * (n + 1)
    choice = [None] * (n + 1)
    dp[0] = 0.0
    for b in range(1, n + 1):
        for a in range(max(0, b - 48), b):
            c, rects = _group_cost_and_rects(mask, r0 + a, r0 + b,
                                             elem_ns, oh)
            if dp[a] + c < dp[b]:
                dp[b] = dp[a] + c
                choice[b] = (a, rects)
    out = []
    b = n
    while b > 0:
        a, rects = choice[b]
        out.extend(rects)
        b = a
    out.reverse()
    return out


def _paint(canvas, rects):
    for (a, b, c, d) in rects:
        canvas[a:b, c:d] = True


def _shift_req(req_canvas, rects, d, axis):
    """src required at rect and rect shifted +d along axis."""
    for (a, b, c, e) in rects:
        req_canvas[a:b, c:e] = True
        if axis == "r":
            req_canvas[a + d:b + d, c:e] = True
        else:
            req_canvas[a:b, c + d:e + d] = True


class BandPlan:
    """Per-band exact-mask plan: blend segments, level groups, base-array
    groups, carry col intervals."""

    def __init__(self, it, carry_ee_cols, carry_oo_cols, next_oo13_cols,
                 carry_pers):
        self.it = it
        y0, y1 = BANDS[it]
        self.y0, self.y1, self.H = y0, y1, y1 - y0
        OB = 32 * it - 14
        self.OB = OB
        req = {n: np.zeros((CANV_R, CANV_C), bool) for n in LEVELS}
        req["ee"] = np.zeros((CANV_R, CANV_C), bool)
        req["oo"] = np.zeros((CANV_R, CANV_C), bool)

        # ---- blends ----
        # ring masks within band rows, in (canvas-row-of-out-row, col):
        # out row R -> blend writes; source level coords = (R+roff, J+coff).
        rows = slice(y0, y1)
        self.blend = {}          # ring -> list of segments
        # segment: (kind, rlo, rhi, clo, chi) kind in {"cp", "act"}
        for r in (4, 3, 2, 1, 0):
            any_m = RING_ANY[r][rows]
            if not any_m.any():
                self.blend[r] = []
                continue
            both_m = RING_BOTH[r][rows]
            segs = []
            if r == 4:
                # unmasked write; don't-care anywhere (later cps fix rest)
                rects = decompose(any_m, TT_ELEM, OP_OH)
                for (a, b, c, d) in rects:
                    segs.append(("s13w", y0 + a, y0 + b, c, d))
            else:
                rects = decompose(any_m, CP_ELEM, OP_OH)
                for (a, b, c, d) in rects:
                    # unconditional interior: cols where all rows in group
                    # are true in BOTH wg masks
                    sub = both_m[a:b, c:d]
                    allin = sub.all(0)
                    ji = np.where(allin)[0]
                    ja = jb = None
                    if len(ji):
                        # largest contiguous all-true run
                        runs = np.split(ji, np.where(np.diff(ji) != 1)[0] + 1)
                        run = max(runs, key=len)
                        if len(run) * (b - a) >= 170:
                            ja, jb = c + int(run[0]), c + int(run[-1]) + 1
                    if ja is None:
                        segs.append(("cp", y0 + a, y0 + b, c, d))
                    else:
                        if ja > c:
                            segs.append(("cp", y0 + a, y0 + b, c, ja))
                        segs.append(("act", y0 + a, y0 + b, ja, jb))
                        if d > jb:
                            segs.append(("cp", y0 + a, y0 + b, jb, d))
            self.blend[r] = segs
            # source requirements (full rects incl. masked cells)
            lvl, roff, coff = BLEND_SRC[r]
            for (_k, rlo, rhi, clo, chi) in segs:
                ra, rb = rlo + roff - OB, rhi + roff - OB
                ca, cb = clo + coff, chi + coff
                assert 0 <= ra and rb <= CANV_R and cb <= CANV_C, (it, r)
                req[lvl][ra:rb, ca:cb] = True

        # ---- levels (reverse topo) ----
        self.groups = {}         # level -> list of canvas rects
        self.carry_pers_out = {}
        self.carry_pers_copy = {}
        for name in REV_ORDER:
            m = req[name]
            if name in PERSIST:
                lo = PERSIST[name]
                ci = carry_pers.get(name)
                if ci is not None:
                    m[32:lo + 32] |= ci
                if it == 0:
                    fresh = m            # no previous band: compute all rows
                else:
                    assert not m[lo + 32:].any(), (it, name)
                    self.carry_pers_out[name] = m[0:lo].copy()
                    cm = m[0:lo]
                    if cm.any():
                        ri = np.where(cm.any(1))[0]
                        self.carry_pers_copy[name] = (
                            int(ri.min()), int(ri.max()) + 1,
                            self._carry_cols(cm))
                    fresh = np.zeros_like(m)
                    fresh[lo:lo + 32] = m[lo:lo + 32]
                m = fresh
            if not m.any():
                self.groups[name] = []
                continue
            rects = decompose(m, TT_ELEM, OP_OH)
            self.groups[name] = rects
            src, d, axis = LEVELS[name]
            _shift_req(req[src], rects, d, axis)

        # ---- ee / oo ----
        # carry-in requirement from next band (rows 32:46 here = next 0:14)
        if carry_ee_cols is not None:
            req["ee"][32:46] |= carry_ee_cols
        if carry_oo_cols is not None:
            req["oo"][32:46] |= carry_oo_cols
        self.req_ee = req["ee"]
        self.req_oo = req["oo"]
        # carry-out requirement to previous band (oo row 13 excluded: it is
        # always rewritten by the oo13 special op, carry content don't-care)
        self.carry_ee = req["ee"][0:14].copy()
        self.carry_oo = req["oo"][0:14].copy()
        self.carry_oo[13] = False
        # oo row 13 special cols
        oi = np.where(req["oo"][13])[0]
        self.oo13 = (int(oi.min()), int(oi.max()) + 1) if len(oi) else None
        # fresh realizations
        ee_fresh = np.zeros_like(req["ee"])
        ee_fresh[14:46] = req["ee"][14:46]
        if it == 7:
            self.ee_groups = []          # memset instead
            self.oo_groups = []
        else:
            force = [16, 18, 22, 26, 30, 34, 38] if it == 0 else []
            self.ee_groups = self._split_rows(ee_fresh, force)
            oo_fresh = np.zeros_like(req["oo"])
            oo_fresh[14:45] = req["oo"][14:45]
            self.oo_groups = self._split_rows(oo_fresh, [])
        # carry copy col intervals (<=2) for rows 0:14
        self.carry_ee_copy = self._carry_cols(self.carry_ee)
        self.carry_oo_copy = self._carry_cols(self.carry_oo)

        # ---- Ew ----
        # Ew row 2j, 2j+1 needed at ee fresh row 14+j cols; canvas [64, C]
        if it == 7:
            self.ew_groups = []
            self.owlast_cols = None
        else:
            ewm = np.zeros((64, CANV_C), bool)
            for (a, b, c, d) in self.ee_groups:
                j0, j1 = a - 14, b - 14
                ewm[2 * j0:2 * j1, c:d] = True
            force = [4, 8, 16, 24, 32, 40, 48] if it == 0 else []
            self.ew_groups = self._split_rows_generic(ewm, force)
            self.owlast_cols = next_oo13_cols

    @staticmethod
    def _split_rows(mask, boundaries):
        rects = []
        bounds = [0] + boundaries + [CANV_R]
        for a, b in zip(bounds, bounds[1:]):
            sub = np.zeros_like(mask)
            sub[a:b] = mask[a:b]
            rects.extend(decompose(sub, TT_ELEM, OP_OH))
        return rects

    @staticmethod
    def _split_rows_generic(mask, boundaries):
        R = mask.shape[0]
        rects = []
        bounds = [0] + [b for b in boundaries if 0 < b < R] + [R]
        for a, b in zip(bounds, bounds[1:]):
            sub = np.zeros_like(mask)
            sub[a:b] = mask[a:b]
            rects.extend(decompose(sub, TT_ELEM, OP_OH))
        return rects

    @staticmethod
    def _carry_cols(mask):
        """rows 0:14 carry mask -> list of (c0, c1) intervals (<=2)."""
        cols = mask.any(0)
        ci = np.where(cols)[0]
        if not len(ci):
            return []
        clo, chi = int(ci.min()), int(ci.max()) + 1
        gaps = np.where(~cols[clo:chi])[0]
        if len(gaps) >= 16:
            runs = np.split(gaps, np.where(np.diff(gaps) != 1)[0] + 1)
            run = max(runs, key=len)
            if len(run) >= 16:
                g0, g1 = clo + int(run[0]), clo + int(run[-1]) + 1
                return [(clo, g0), (g1, chi)]
        return [(clo, chi)]


def _build_plans():
    plans = [None] * NBANDS
    carry_ee = carry_oo = None
    next_oo13 = None
    carry_pers = {}
    for it in range(NBANDS - 1, -1, -1):
        p = BandPlan(it, carry_ee, carry_oo, next_oo13, carry_pers)
        plans[it] = p
        carry_ee, carry_oo = p.carry_ee, p.carry_oo
        carry_pers = p.carry_pers_out
        next_oo13 = p.oo13
    return plans


PLANS = _build_plans()

# ---- validate coverage: realized(src) must cover all reads ----


def _validate_plans():
    for it, p in enumerate(PLANS):
        real = {}
        for name in list(LEVELS) + ["ee", "oo"]:
            cv = np.zeros((CANV_R, CANV_C), bool)
            if name == "ee":
                _paint(cv, p.ee_groups)
                cv[0:14] = True if it > 0 else False
                if it == 0:
                    cv[0:14] = True     # memset
                if it == 7:
                    cv[14:46] = True    # memset
                # carry rows realized iff prev band realized 32:46 there —
                # checked via carry_ee ⊆ prev realized below
            elif name == "oo":
                _paint(cv, p.oo_groups)
                cv[0:14] = True
                if p.oo13 is not None:
                    cv[13, p.oo13[0]:p.oo13[1]] = True
                if it == 7:
                    cv[14:46] = True
            else:
                _paint(cv, p.groups.get(name, []))
                if name in PERSIST and it > 0:
                    cv[0:PERSIST[name]] = True   # carried (checked below)
            real[name] = cv
        # each level's reads covered by src realization
        for name in REV_ORDER:
            rects = p.groups.get(name, [])
            if not rects:
                continue
            src, d, axis = LEVELS[name]
            need = np.zeros((CANV_R, CANV_C), bool)
            _shift_req(need, rects, d, axis)
            assert not (need & ~real[src]).any(), (it, name, src)
        # blend reads covered
        for r, segs in p.blend.items():
            lvl, roff, coff = BLEND_SRC[r]
            for (_k, rlo, rhi, clo, chi) in segs:
                ra, rb = rlo + roff - p.OB, rhi + roff - p.OB
                sub = real[lvl][ra:rb, clo + coff:chi + coff]
                assert sub.all(), (it, r, _k)
        # carry feasibility: this band's carry req ⊆ prev band's realized
        if it > 0:
            prev = PLANS[it - 1]
            pr = np.zeros((CANV_R, CANV_C), bool)
            _paint(pr, prev.ee_groups)
            if it - 1 == 0:
                pr[0:14] = True
            assert not (p.carry_ee & ~pr[32:46]).any(), (it, "carry_ee")
            po = np.zeros((CANV_R, CANV_C), bool)
            _paint(po, prev.oo_groups)
            if prev.oo13 is not None:
                po[13, prev.oo13[0]:prev.oo13[1]] = True
            assert not (p.carry_oo & ~po[32:46]).any(), (it, "carry_oo")
            for name, lo in PERSIST.items():
                cn = p.carry_pers_out.get(name)
                if cn is None or not cn.any():
                    continue
                pm = np.zeros((CANV_R, CANV_C), bool)
                _paint(pm, prev.groups.get(name, []))
                assert not (cn & ~pm[32:lo + 32]).any(), (it, name)


_validate_plans()

# ---- tile sizing ----

LV_TAG = dict(a2="tP", a4="tP", a8="tP", v13="tP",
              s2="tQ", s8="tQ", s4="tS4", u7="tT", w10="tT",
              s7="tS7", a2o="tPo", a4o="tPo", a8o="tPo",
              s2o="tQo", s8o="tQo", s4o="tS4o", s10="tS10")


def _bbox(rects):
    r0 = min(a for a, b, c, d in rects)
    r1 = max(b for a, b, c, d in rects)
    c0 = min(c for a, b, c, d in rects)
    c1 = max(d for a, b, c, d in rects)
    return r0, r1, c0, c1


LV_ORIGIN = []        # per band: level -> (r0, c0) canvas origin of tile
TAG_MAX = {}
for _p in PLANS:
    org = {}
    for _n in LEVELS:
        rects = _p.groups.get(_n, [])
        if not rects:
            continue
        r0, r1, c0, c1 = _bbox(rects)
        org[_n] = (r0, c0)
        if _n == "s13" or _n in PERSIST:
            continue
        t = LV_TAG[_n]
        sz = TAG_MAX.get(t, (0, 0))
        TAG_MAX[t] = (max(sz[0], r1 - r0), max(sz[1], c1 - c0))
    LV_ORIGIN.append(org)

PERS_DIM = {}
for _n, _lo in PERSIST.items():
    _w = 1
    for _p in PLANS:
        for (_a, _b, _c, _d) in _p.groups.get(_n, []):
            _w = max(_w, _d)
    PERS_DIM[_n] = (_lo + 32, _w)

# ---- blend mask strips ----


def _strip_mask(it, rlo, rhi, clo, chi, ring):
    y0 = PLANS[it].y0
    n0 = (_R0 == ring)[rlo:rhi, clo:chi].astype(np.uint8)
    n1 = (_R1[:, 0:CEN] if False else (_R1 == ring))[rlo:rhi, clo:chi] \
        .astype(np.uint8)
    m = np.zeros((128, rhi - rlo, chi - clo), np.uint8)
    m[0:64] = n0[None]
    m[64:128] = n1[None]
    return m


def _build_strips():
    blobs, bands, offs = [], [], []
    pos = 0
    for it, p in enumerate(PLANS):
        start = pos
        ent = {}
        for r in (3, 2, 1, 0):
            for gi, seg in enumerate(p.blend.get(r, [])):
                kind, rlo, rhi, clo, chi = seg
                if kind != "cp":
                    continue
                m = _strip_mask(it, rlo, rhi, clo, chi, r)
                nr, nc = m.shape[1], m.shape[2]
                ent[(r, gi)] = (pos - start, nr, nc)
                blobs.append(np.ascontiguousarray(m).reshape(128, -1))
                pos += nr * nc
        offs.append(ent)
        bands.append((start, pos - start))
    blob = (np.concatenate(blobs, 1) if blobs
            else np.zeros((128, 1), np.uint8))
    return blob, bands, offs


STRIP_BLOB, STRIP_BANDS, STRIP_OFFS = _build_strips()
STRIP_MAX = max(max(sz for _, sz in STRIP_BANDS), 1)

# ---------------- shared band program ----------------


def _emit_band(be, it):
    p = PLANS[it]
    y0, y1, H, OB = p.y0, p.y1, p.H, p.OB
    ee, oo, Ew, ol = be.ee, be.oo, be.Ew, be.owlast

    # ---- input DMA ----
    if 0 < it < 7:
        az = be.azeo()
        be.dma_band(it, az)
    elif it == 0:
        az = be.azeo()
        for r0, r1 in ((0, 4), (4, 8), (8, 16), (16, 24), (24, 32),
                       (32, 40), (40, 48), (48, 64)):
            be.dma_rows(az, r0, r1)

    # ---- Ew build ----
    if it < 7:
        for (a, b, c, d) in p.ew_groups:
            be.max2(Ew[:, a:b, c:d], az[:, a:b, c:d], az[:, a:b, 126 + c:126 + d])

    # ---- carries ----
    if it > 0:
        for (c0, c1) in p.carry_ee_copy:
            be.gcopy(ee[:, 0:14, c0:c1], ee[:, 32:46, c0:c1])
        for (c0, c1) in p.carry_oo_copy:
            be.gcopy(oo[:, 0:14, c0:c1], oo[:, 32:46, c0:c1])
        for name in PERSIST:
            cc = p.carry_pers_copy.get(name)
            if cc is None:
                continue
            ra, rb, ivs = cc
            t = be.pers_lv(name)
            for (c0, c1) in ivs:
                be.gcopy(t[:, ra:rb, c0:c1], t[:, ra + 32:rb + 32, c0:c1])
    else:
        be.memset(ee[:, 0:14, :], NEG)
        be.memset(oo[:, 0:14, :], NEG)

    # ---- ee/oo fresh ----
    if it == 7:
        be.memset(ee[:, 14:46, :], NEG)
        be.memset(oo[:, 14:46, :], NEG)
        if p.oo13 is not None:
            s0, s1 = p.oo13
            be.scopy(oo[:, 13:14, s0:s1], ol[:, 0:1, s0:s1])
    else:
        for (a, b, c, d) in p.ee_groups:
            j0, j1 = a - 14, b - 14
            be.max2(ee[:, a:b, c:d], Ew[:, 2 * j0:2 * j1:2, c:d],
                    Ew[:, 2 * j0 + 1:2 * j1:2, c:d])
        if p.oo13 is not None:
            s0, s1 = p.oo13
            be.max2(oo[:, 13:14, s0:s1], az[:, 0:1, 126 + s0:126 + s1],
                    az[:, 0:1, 1 + s0:1 + s1])
            if it > 0:
                be.max2(oo[:, 13:14, s0:s1], oo[:, 13:14, s0:s1],
                        ol[:, 0:1, s0:s1])
        for (a, b, c, d) in p.oo_groups:
            j0, j1 = a - 14, b - 14
            be.max2(oo[:, a:b, c:d],
                    az[:, 2 * j0 + 1:2 * j1 + 1:2, 126 + c:126 + d],
                    az[:, 2 * j0 + 1:2 * j1 + 1:2, 1 + c:1 + d])
            be.max2(oo[:, a:b, c:d], oo[:, a:b, c:d],
                    az[:, 2 * j0 + 2:2 * j1 + 2:2, 126 + c:126 + d])
            be.max2(oo[:, a:b, c:d], oo[:, a:b, c:d],
                    az[:, 2 * j0 + 2:2 * j1 + 2:2, 1 + c:1 + d])
        if p.owlast_cols is not None:
            g0, g1 = p.owlast_cols
            be.max2(ol[:, 0:1, g0:g1], az[:, 63:64, 126 + g0:126 + g1],
                    az[:, 63:64, 1 + g0:1 + g1])

    # ---- pyramid ----
    out16 = be.out16()
    tiles = {"ee": (ee, 0, 0), "oo": (oo, 0, 0)}
    for _n in PERSIST:
        tiles[_n] = (be.pers_lv(_n), 0, 0)
    org = LV_ORIGIN[it]

    def src_ap(st, sr0, sc0, a, b, c, d):
        return st[:, a - sr0:b - sr0, c - sc0:d - sc0]

    emit_order = ["a2", "a2o", "s2", "s2o", "a4", "a4o", "s4", "s4o",
                  "a8", "a8o", "u7", "s8", "s8o", "s7", "w10", "v13",
                  "s10", "s13"]
    for name in emit_order:
        rects = p.groups.get(name, [])
        if not rects:
            continue
        srcn, dsh, axis = LEVELS[name]
        st, sr0, sc0 = tiles[srcn]
        if name == "s13":
            # write directly into out16: out row = canvas row + OB + 6
            for (a, b, c, d) in rects:
                ra, rb = a + OB + 6 - y0, b + OB + 6 - y0
                dst = out16[:, ra:rb, c:d]
                be.max2(dst, src_ap(st, sr0, sc0, a, b, c, d),
                        src_ap(st, sr0, sc0, a, b, c + dsh, d + dsh))
            continue
        if name in PERSIST:
            t = be.pers_lv(name)
            r0 = c0 = 0
        else:
            r0, c0 = org[name]
            t = be.lv(name)
        tiles[name] = (t, r0, c0)
        for (a, b, c, d) in rects:
            dst = t[:, a - r0:b - r0, c - c0:d - c0]
            if axis == "r":
                be.max2(dst, src_ap(st, sr0, sc0, a, b, c, d),
                        src_ap(st, sr0, sc0, a + dsh, b + dsh, c, d))
            else:
                be.max2(dst, src_ap(st, sr0, sc0, a, b, c, d),
                        src_ap(st, sr0, sc0, a, b, c + dsh, d + dsh))

    # ---- blends (rings 3,2,1,0 after s13 write) ----
    if any(k[0] in (0, 1, 2, 3) for k in STRIP_OFFS[it]):
        be.dma_strip(it)
    for r in (3, 2, 1, 0):
        lvl, roff, coff = BLEND_SRC[r]
        st, sr0, sc0 = tiles.get(lvl, (None, 0, 0))
        for gi, seg in enumerate(p.blend.get(r, [])):
            kind, rlo, rhi, clo, chi = seg
            ra, rb = rlo + roff - OB, rhi + roff - OB
            data = st[:, ra - sr0:rb - sr0,
                      clo + coff - sc0:chi + coff - sc0]
            dst = out16[:, rlo - y0:rhi - y0, clo:chi]
            if kind == "cp":
                be.cp(dst, be.strip_ap(it, (r, gi)), data)
            else:
                be.acopy(dst, data)

    be.dma_out(it, out16)


def _emit_program(be):
    for it in range(NBANDS):
        _emit_band(be, it)


def prep_input(x1):
    """x1 [C, 448, 448] fp32 -> [128, 448, 250] fp16 parity-split blob.
    Per (wg*64+c, row): [pad6 | zE (119) | pad7 | zO (118)], wg1 mirrored,
    NEG pads.  Pure layout marshalling (cast/reorder/pad), no arithmetic."""
    xz = np.full((2, C, IN, 250), NEG, np.float16)
    xz[0, :, :, 6:125] = x1[:, :, 0:237:2]       # zE0[e]=x[2e-12]
    xz[0, :, :, 132:250] = x1[:, :, 1:236:2]     # zO0[e]=x[2e-11]
    xz[1, :, :, 6:125] = x1[:, :, 447:209:-2]    # zE1[e]=x[459-2e]
    xz[1, :, :, 132:250] = x1[:, :, 446:210:-2]  # zO1[e]=x[458-2e]
    return np.ascontiguousarray(xz.reshape(128, IN, 250))


# ---------------- numpy backend (validation) ----------------


class NumpyBE:
    def __init__(self, x):
        self.xz = prep_input(x).astype(np.float32)
        f32 = np.float32
        self._azeo = np.full((128, 64, 250), np.nan, f32)
        self.Ew = np.full((128, 64, EOW), np.nan, f32)
        self.owlast = np.full((128, 1, EOW), np.nan, f32)
        self.ee = np.full((128, 46, EOW), np.nan, f32)
        self.oo = np.full((128, 46, EOW), np.nan, f32)
        self._pers = {n: np.full((128, r, w), np.nan, f32)
                      for n, (r, w) in PERS_DIM.items()}
        self.y = np.full((128, OUT, OW), np.nan, f32)
        self._out = None

    def azeo(self):
        return self._azeo

    def lv(self, name):
        nr, nc = TAG_MAX[LV_TAG[name]]
        return np.full((128, nr, nc), np.nan, np.float32)

    def pers_lv(self, name):
        return self._pers[name]

    def out16(self):
        self._out = np.full((128, 32, OW), np.nan, np.float32)
        return self._out

    def memset(self, ap, v):
        ap[...] = v

    def max2(self, d, a, b):
        assert d.shape == a.shape == b.shape, (d.shape, a.shape, b.shape)
        np.maximum(a, b, out=d)

    def scopy(self, d, s):
        d[...] = s

    acopy = scopy
    gcopy = scopy

    def cp(self, out, mask, data):
        assert out.shape == mask.shape == data.shape
        out[...] = np.where(mask != 0, data, out)

    def dma_band(self, it, az):
        r0 = 64 * it
        az[:, :, :] = self.xz[:, r0:r0 + 64, :]

    def dma_rows(self, az, a, b):
        az[:, a:b, :] = self.xz[:, a:b, :]

    def dma_strip(self, it):
        pass

    def strip_ap(self, it, key):
        start, _ = STRIP_BANDS[it]
        off, nr, nc = STRIP_OFFS[it][key]
        return STRIP_BLOB[:, start + off:start + off + nr * nc].reshape(
            128, nr, nc)

    def dma_out(self, it, out16):
        p = PLANS[it]
        self.y[:, p.y0:p.y1, :] = out16[:, 0:p.H, :]


def numpy_kernel(x1):
    """x1: [64, 448, 448] -> [64, 224, 224] (fp32, exact clip semantics)."""
    be = NumpyBE(x1)
    _emit_program(be)
    yw = be.y
    assert not np.isnan(yw).any(), "uncovered output pixels"
    out = np.empty((C, OUT, OUT), np.float32)
    out[:, :, 0:OW] = yw[0:64]
    out[:, :, OW:OUT] = yw[64:128, :, ::-1]
    return out


# ---------------- bass backend ----------------


def split_multi_waits(nc):
    """walrus CoreV3Gen accepts at most 1 sync-wait per instruction; Tile's
    tail drains can carry 2+.  Peel extras onto preceding NoOps."""
    n = 0
    for fn in nc.m.functions:
        for bb in fn.blocks:
            insts = list(bb.instructions)
            out = []
            for ins in insts:
                si = getattr(ins, "sync_info", None)
                if si is not None and len(si.on_wait) > 1:
                    waits = list(si.on_wait)
                    for k, w in enumerate(waits[:-1]):
                        nop = mybir.InstNoOp(
                            name=f"{ins.name}-waitsplit{k}",
                            engine=ins.engine, ins=[], outs=[])
                        nop.sync_info = mybir.SyncInfo(
                            on_wait=[w], on_update=[])
                        out.append(nop)
                        n += 1
                    ins.sync_info = mybir.SyncInfo(
                        on_wait=[waits[-1]], on_update=list(si.on_update))
                out.append(ins)
            if n:
                bb.instructions = out
    return n


class BassBE:
    def __init__(self, nc, pools, x, y, strips):
        self.nc = nc
        self.x = x
        self.y = y
        self.strips = strips
        pers, self.lvpool, self.iop, self.chpool, self.strippool = pools
        self.Ew = pers.tile([128, 64, EOW], DT, tag="Ew")
        self.owlast = pers.tile([128, 1, EOW], DT, tag="owlast")
        self.ee = pers.tile([128, 46, EOW], DT, tag="ee")
        self.oo = pers.tile([128, 46, EOW], DT, tag="oo")
        self.pers_tiles = {
            n: pers.tile([128, r, w], DT, tag=f"pers_{n}", name=f"pers_{n}")
            for n, (r, w) in PERS_DIM.items()}
        self._strip = None

    def azeo(self):
        return self.chpool.tile([128, 64, 250], DT, tag="azeo", name="azeo")

    def lv(self, name):
        nr, nc_ = TAG_MAX[LV_TAG[name]]
        return self.lvpool.tile([128, nr, nc_], DT, tag=LV_TAG[name],
                                name=f"lv_{name}")

    def pers_lv(self, name):
        return self.pers_tiles[name]

    def out16(self):
        return self.iop.tile([128, 32, OW], DT, tag="out16", name="out16")

    def memset(self, ap, v):
        self.nc.gpsimd.memset(ap, v)

    def max2(self, d, a, b):
        self.nc.vector.tensor_tensor(d, a, b, MX)

    def scopy(self, d, s):
        self.nc.vector.tensor_scalar_max(d, s, NEG)

    def acopy(self, d, s):
        self.nc.scalar.copy(d, s)

    gcopy = acopy

    def cp(self, out, mask, data):
        self.nc.vector.copy_predicated(out, mask, data)

    def dma_band(self, it, az):
        r0 = 64 * it
        self.nc.sync.dma_start(az[:, :, :], self.x[:, r0:r0 + 64, :])

    def dma_rows(self, az, a, b):
        self.nc.sync.dma_start(az[:, a:b, :], self.x[:, a:b, :])

    def dma_strip(self, it):
        start, sz = STRIP_BANDS[it]
        self._strip = self.strippool.tile([128, STRIP_MAX], mybir.dt.uint8,
                                          tag="strip", name="strip")
        self.nc.sync.dma_start(self._strip[:, 0:sz],
                               self.strips[:, start:start + sz])

    def strip_ap(self, it, key):
        off, nr, nc_ = STRIP_OFFS[it][key]
        return self._strip[:, off:off + nr * nc_].rearrange(
            "p (r c) -> p r c", c=nc_)

    def dma_out(self, it, out16):
        p = PLANS[it]
        self.nc.sync.dma_start(self.y[:, p.y0:p.y1, :],
                               out16[:, 0:p.H, :])


def _emit_kernel(nc: bass.Bass):
    x = nc.dram_tensor("x", [128, IN, 250], DT, kind="ExternalInput")
    y = nc.dram_tensor("y", [128, OUT, OW], DT, kind="ExternalOutput")
    strips = nc.inline_tensor(STRIP_BLOB, name="mstrips")

    with TileContext(nc) as tc:
        with tc.tile_pool(name="pp", bufs=1) as pers, \
             tc.tile_pool(name="lv", bufs=1) as lvpool, \
             tc.tile_pool(name="io", bufs=1) as iop, \
             tc.tile_pool(name="ch", bufs=2) as chpool, \
             tc.tile_pool(name="st", bufs=1) as strippool:
            be = BassBE(nc, (pers, lvpool, iop, chpool, strippool), x, y, strips)
            _emit_program(be)
    return nc


_CACHED = {}


def _get_nc():
    if "nc" not in _CACHED:
        nc = bass.Bass()
        _emit_kernel(nc)
        split_multi_waits(nc)
        _CACHED["nc"] = nc
    return _CACHED["nc"]


def kernel(x: np.ndarray) -> np.ndarray:
    nc = _get_nc()
    in_maps = [{"x": prep_input(x[b].astype(np.float32))}
               for b in range(B)]
    res = run_bass_kernel_spmd(nc, in_maps, core_ids=list(range(B)))
    out = np.empty((B, C, OUT, OUT), np.float32)
    for b, r in enumerate(res.results):
        yw = r["y"].astype(np.float32)      # [128, 224, 112]
        out[b, :, :, 0:OW] = yw[0:64]
        out[b, :, :, OW:OUT] = yw[64:128, :, ::-1]
    return out


# revision 23
# speedup vs baseline: 1.1606x; 1.0087x over previous
"""Trainium2 Bass kernel for CenterDependentPool2D (v4).

Input  x: (8, 64, 448, 448) fp32  ->  Output: (8, 64, 224, 224) fp32.

Per core = one batch element.  Partition p = c + 64*wg: channel c, wg 0 =
out cols 0..111 (natural j), wg 1 = out cols 223..112 (MIRRORED local j).
Host prep emits a parity-split fp16 blob; device computes E/O pair arrays
and shifted-max doubling pyramids for the 5 ring windows (k in
{2,8,14,20,26}), blending by ring masks.

v4 over v3: per-band EXACT needed masks are backward-propagated through
the pyramid DAG and each level is emitted as a DP-chosen set of row-group
rectangles (tight col bounds, optional gap split) instead of a single
bounding-box hull; blend rectangles get unconditional-interior splits
(interior -> Activation-engine copy, boundary strips -> masked
copy_predicated on DVE); carries are column-gated; DMA uses flattened
[128, ...] tensors for bigger descriptors.
"""

import numpy as np

import concourse.bass as bass
import concourse.mybir as mybir
from concourse.tile import TileContext
from concourse.bass_utils import run_bass_kernel_spmd

# ---------------- problem constants ----------------
B, C, IN, OUT = 8, 64, 448, 224
CEN = 112
OW = 112
NEG = -30000.0
RADII = (60, 75, 90, 105)
DT = mybir.dt.float16
MX = mybir.AluOpType.max

# out-row bands: [0,24), [24,56), ..., [184,216), [216,224)
BANDS = [(0, 24)] + [(24 + 32 * k, 56 + 32 * k) for k in range(6)] \
    + [(216, 224)]
NBANDS = len(BANDS)

CANV_R, CANV_C = 46, 132          # per-band level canvas (abs rows OB..OB+46)
EOW = 125                         # Ew/Ow/ee/oo tile width (cols used <= 124)

# DP cost constants (ns)
TT_ELEM = 0.52
CP_ELEM = 0.90
OP_OH = 45.0
ACT_ELEM = 0.83
ACT_OH = 280.0

# ---------------- static geometry ----------------

_yy, _xx = np.mgrid[0:OUT, 0:OUT]
_D2 = (_yy - CEN) ** 2 + (_xx - CEN) ** 2
NESTED = np.stack([(_D2 < R * R) for R in RADII])
RING_ID = 4 - NESTED.sum(0)


def _localize(a):
    return a[:, 0:CEN], a[:, ::-1][:, 0:CEN]


_R0, _R1 = _localize(RING_ID)
# union/both ring-cell masks in localized coords
RING_ANY = [np.asarray((_R0 == r) | (_R1 == r)) for r in range(5)]
RING_BOTH = [np.asarray((_R0 == r) & (_R1 == r)) for r in range(5)]

# blend source per ring: (level, row_off, col_off): out (R, J) reads
# level[R + roff, J + coff]
BLEND_SRC = {4: ("s13", -6, 0), 3: ("s10", -5, 1), 2: ("s7", -3, 3),
             1: ("s4o", -2, 4), 0: ("ee", 0, 6)}

# pyramid DAG: level -> (src, shift, axis)
LEVELS = {
    "a2": ("ee", 1, "r"), "s2": ("a2", 1, "c"),
    "a4": ("s2", 2, "r"), "s4": ("a4", 2, "c"),
    "a8": ("s4", 4, "r"), "s8": ("a8", 4, "c"),
    "v13": ("s8", 5, "r"), "s13": ("v13", 5, "c"),
    "u7": ("s4", 3, "r"), "s7": ("u7", 3, "c"),
    "a2o": ("oo", 1, "r"), "s2o": ("a2o", 1, "c"),
    "a4o": ("s2o", 2, "r"), "s4o": ("a4o", 2, "c"),
    "a8o": ("s4o", 4, "r"), "s8o": ("a8o", 4, "c"),
    "w10": ("s8o", 2, "r"), "s10": ("w10", 2, "c"),
}
# realization order: consumers before producers
REV_ORDER = ["s13", "v13", "s8", "s7", "a8", "u7", "s4", "a4", "s2", "a2",
             "s10", "w10", "s8o", "a8o", "s4o", "a4o", "s2o", "a2o"]

# persistent mid-chain levels: level -> fresh-window start lo (fresh rows
# [lo, lo+32) per band; rows [0, lo) carried from the previous band).  The
# phases fit the 46-row canvas exactly (a2/s2 pull down to rows [11,45),
# reading ee rows up to 45).
PERSIST = {"s2": 13, "s4": 11, "s8": 7, "s2o": 12, "s4o": 10, "s8o": 6}


def decompose(mask, elem_ns=TT_ELEM, oh=OP_OH):
    """mask: bool [R, C] -> list of rects (r0, r1, c0, c1) covering mask.
    DP over row boundaries; per group tight col bbox, optional split into
    2 col intervals at the largest internal gap."""
    R, Cc = mask.shape
    rows_any = mask.any(1)
    rects = []
    r = 0
    while r < R:
        if not rows_any[r]:
            r += 1
            continue
        e = r
        while e < R and rows_any[e]:
            e += 1
        rects.extend(_dp_run(mask, r, e, elem_ns, oh))
        r = e
    return rects


def _group_cost_and_rects(mask, a, b, elem_ns, oh):
    sub = mask[a:b]
    cols = sub.any(0)
    ci = np.where(cols)[0]
    clo, chi = int(ci.min()), int(ci.max()) + 1
    nr = b - a
    best = (nr * (chi - clo) * elem_ns + oh, [(a, b, clo, chi)])
    # largest internal gap
    gaps = np.where(~cols[clo:chi])[0]
    if len(gaps):
        # find longest run of gaps
        runs = np.split(gaps, np.where(np.diff(gaps) != 1)[0] + 1)
        run = max(runs, key=len)
        g0, g1 = clo + int(run[0]), clo + int(run[-1]) + 1
        c2 = (nr * ((chi - clo) - (g1 - g0))) * elem_ns + 2 * oh
        if c2 < best[0]:
            best = (c2, [(a, b, clo, g0), (a, b, g1, chi)])
    return best


def _dp_run(mask, r0, r1, elem_ns, oh):
    n = r1 - r0
    INF = float("inf")
    dp = [INF] * (n + 1)
    choice = [None] * (n + 1)
    dp[0] = 0.0
    for b in range(1, n + 1):
        for a in range(max(0, b - 48), b):
            c, rects = _group_cost_and_rects(mask, r0 + a, r0 + b,
                                             elem_ns, oh)
            if dp[a] + c < dp[b]:
                dp[b] = dp[a] + c
                choice[b] = (a, rects)
    out = []
    b = n
    while b > 0:
        a, rects = choice[b]
        out.extend(rects)
        b = a
    out.reverse()
    return out


def _paint(canvas, rects):
    for (a, b, c, d) in rects:
        canvas[a:b, c:d] = True


def _shift_req(req_canvas, rects, d, axis):
    """src required at rect and rect shifted +d along axis."""
    for (a, b, c, e) in rects:
        req_canvas[a:b, c:e] = True
        if axis == "r":
            req_canvas[a + d:b + d, c:e] = True
        else:
            req_canvas[a:b, c + d:e + d] = True


class BandPlan:
    """Per-band exact-mask plan: blend segments, level groups, base-array
    groups, carry col intervals."""

    def __init__(self, it, carry_ee_cols, carry_oo_cols, next_oo13_cols,
                 carry_pers):
        self.it = it
        y0, y1 = BANDS[it]
        self.y0, self.y1, self.H = y0, y1, y1 - y0
        OB = 32 * it - 14
        self.OB = OB
        req = {n: np.zeros((CANV_R, CANV_C), bool) for n in LEVELS}
        req["ee"] = np.zeros((CANV_R, CANV_C), bool)
        req["oo"] = np.zeros((CANV_R, CANV_C), bool)

        # ---- blends ----
        # ring masks within band rows, in (canvas-row-of-out-row, col):
        # out row R -> blend writes; source level coords = (R+roff, J+coff).
        rows = slice(y0, y1)
        self.blend = {}          # ring -> list of segments
        # segment: (kind, rlo, rhi, clo, chi) kind in {"cp", "act"}
        for r in (4, 3, 2, 1, 0):
            any_m = RING_ANY[r][rows]
            if not any_m.any():
                self.blend[r] = []
                continue
            both_m = RING_BOTH[r][rows]
            segs = []
            if r == 4:
                # unmasked write; don't-care anywhere (later cps fix rest)
                rects = decompose(any_m, TT_ELEM, OP_OH)
                for (a, b, c, d) in rects:
                    segs.append(("s13w", y0 + a, y0 + b, c, d))
            else:
                rects = decompose(any_m, CP_ELEM, OP_OH)
                for (a, b, c, d) in rects:
                    # unconditional interior: cols where all rows in group
                    # are true in BOTH wg masks
                    sub = both_m[a:b, c:d]
                    allin = sub.all(0)
                    ji = np.where(allin)[0]
                    ja = jb = None
                    if len(ji):
                        # largest contiguous all-true run
                        runs = np.split(ji, np.where(np.diff(ji) != 1)[0] + 1)
                        run = max(runs, key=len)
                        if len(run) * (b - a) >= 170:
                            ja, jb = c + int(run[0]), c + int(run[-1]) + 1
                    if ja is None:
                        segs.append(("cp", y0 + a, y0 + b, c, d))
                    else:
                        if ja > c:
                            segs.append(("cp", y0 + a, y0 + b, c, ja))
                        segs.append(("act", y0 + a, y0 + b, ja, jb))
                        if d > jb:
                            segs.append(("cp", y0 + a, y0 + b, jb, d))
            self.blend[r] = segs
            # source requirements (full rects incl. masked cells)
            lvl, roff, coff = BLEND_SRC[r]
            for (_k, rlo, rhi, clo, chi) in segs:
                ra, rb = rlo + roff - OB, rhi + roff - OB
                ca, cb = clo + coff, chi + coff
                assert 0 <= ra and rb <= CANV_R and cb <= CANV_C, (it, r)
                req[lvl][ra:rb, ca:cb] = True

        # ---- levels (reverse topo) ----
        self.groups = {}         # level -> list of canvas rects
        self.carry_pers_out = {}
        self.carry_pers_copy = {}
        for name in REV_ORDER:
            m = req[name]
            if name in PERSIST:
                lo = PERSIST[name]
                ci = carry_pers.get(name)
                if ci is not None:
                    m[32:lo + 32] |= ci
                if it == 0:
                    fresh = m            # no previous band: compute all rows
                else:
                    assert not m[lo + 32:].any(), (it, name)
                    self.carry_pers_out[name] = m[0:lo].copy()
                    cm = m[0:lo]
                    if cm.any():
                        ri = np.where(cm.any(1))[0]
                        self.carry_pers_copy[name] = (
                            int(ri.min()), int(ri.max()) + 1,
                            self._carry_cols(cm))
                    fresh = np.zeros_like(m)
                    fresh[lo:lo + 32] = m[lo:lo + 32]
                m = fresh
            if not m.any():
                self.groups[name] = []
                continue
            rects = decompose(m, TT_ELEM, OP_OH)
            self.groups[name] = rects
            src, d, axis = LEVELS[name]
            _shift_req(req[src], rects, d, axis)

        # ---- ee / oo ----
        # carry-in requirement from next band (rows 32:46 here = next 0:14)
        if carry_ee_cols is not None:
            req["ee"][32:46] |= carry_ee_cols
        if carry_oo_cols is not None:
            req["oo"][32:46] |= carry_oo_cols
        self.req_ee = req["ee"]
        self.req_oo = req["oo"]
        # carry-out requirement to previous band (oo row 13 excluded: it is
        # always rewritten by the oo13 special op, carry content don't-care)
        self.carry_ee = req["ee"][0:14].copy()
        self.carry_oo = req["oo"][0:14].copy()
        self.carry_oo[13] = False
        # oo row 13 special cols
        oi = np.where(req["oo"][13])[0]
        self.oo13 = (int(oi.min()), int(oi.max()) + 1) if len(oi) else None
        # fresh realizations
        ee_fresh = np.zeros_like(req["ee"])
        ee_fresh[14:46] = req["ee"][14:46]
        if it == 7:
            self.ee_groups = []          # memset instead
            self.oo_groups = []
        else:
            force = [16, 18, 22, 26, 30, 34, 38] if it == 0 else []
            self.ee_groups = self._split_rows(ee_fresh, force)
            oo_fresh = np.zeros_like(req["oo"])
            oo_fresh[14:45] = req["oo"][14:45]
            self.oo_groups = self._split_rows(oo_fresh, [])
        # carry copy col intervals (<=2) for rows 0:14
        self.carry_ee_copy = self._carry_cols(self.carry_ee)
        self.carry_oo_copy = self._carry_cols(self.carry_oo)

        # ---- Ew ----
        # Ew row 2j, 2j+1 needed at ee fresh row 14+j cols; canvas [64, C]
        if it == 7:
            self.ew_groups = []
            self.owlast_cols = None
        else:
            ewm = np.zeros((64, CANV_C), bool)
            for (a, b, c, d) in self.ee_groups:
                j0, j1 = a - 14, b - 14
                ewm[2 * j0:2 * j1, c:d] = True
            force = [4, 8, 16, 24, 32, 40, 48] if it == 0 else []
            self.ew_groups = self._split_rows_generic(ewm, force)
            self.owlast_cols = next_oo13_cols

    @staticmethod
    def _split_rows(mask, boundaries):
        rects = []
        bounds = [0] + boundaries + [CANV_R]
        for a, b in zip(bounds, bounds[1:]):
            sub = np.zeros_like(mask)
            sub[a:b] = mask[a:b]
            rects.extend(decompose(sub, TT_ELEM, OP_OH))
        return rects

    @staticmethod
    def _split_rows_generic(mask, boundaries):
        R = mask.shape[0]
        rects = []
        bounds = [0] + [b for b in boundaries if 0 < b < R] + [R]
        for a, b in zip(bounds, bounds[1:]):
            sub = np.zeros_like(mask)
            sub[a:b] = mask[a:b]
            rects.extend(decompose(sub, TT_ELEM, OP_OH))
        return rects

    @staticmethod
    def _carry_cols(mask):
        """rows 0:14 carry mask -> list of (c0, c1) intervals (<=2)."""
        cols = mask.any(0)
        ci = np.where(cols)[0]
        if not len(ci):
            return []
        clo, chi = int(ci.min()), int(ci.max()) + 1
        gaps = np.where(~cols[clo:chi])[0]
        if len(gaps) >= 16:
            runs = np.split(gaps, np.where(np.diff(gaps) != 1)[0] + 1)
            run = max(runs, key=len)
            if len(run) >= 16:
                g0, g1 = clo + int(run[0]), clo + int(run[-1]) + 1
                return [(clo, g0), (g1, chi)]
        return [(clo, chi)]


def _build_plans():
    plans = [None] * NBANDS
    carry_ee = carry_oo = None
    next_oo13 = None
    carry_pers = {}
    for it in range(NBANDS - 1, -1, -1):
        p = BandPlan(it, carry_ee, carry_oo, next_oo13, carry_pers)
        plans[it] = p
        carry_ee, carry_oo = p.carry_ee, p.carry_oo
        carry_pers = p.carry_pers_out
        next_oo13 = p.oo13
    return plans


PLANS = _build_plans()

# ---- validate coverage: realized(src) must cover all reads ----


def _validate_plans():
    for it, p in enumerate(PLANS):
        real = {}
        for name in list(LEVELS) + ["ee", "oo"]:
            cv = np.zeros((CANV_R, CANV_C), bool)
            if name == "ee":
                _paint(cv, p.ee_groups)
                cv[0:14] = True if it > 0 else False
                if it == 0:
                    cv[0:14] = True     # memset
                if it == 7:
                    cv[14:46] = True    # memset
                # carry rows realized iff prev band realized 32:46 there —
                # checked via carry_ee ⊆ prev realized below
            elif name == "oo":
                _paint(cv, p.oo_groups)
                cv[0:14] = True
                if p.oo13 is not None:
                    cv[13, p.oo13[0]:p.oo13[1]] = True
                if it == 7:
                    cv[14:46] = True
            else:
                _paint(cv, p.groups.get(name, []))
                if name in PERSIST and it > 0:
                    cv[0:PERSIST[name]] = True   # carried (checked below)
            real[name] = cv
        # each level's reads covered by src realization
        for name in REV_ORDER:
            rects = p.groups.get(name, [])
            if not rects:
                continue
            src, d, axis = LEVELS[name]
            need = np.zeros((CANV_R, CANV_C), bool)
            _shift_req(need, rects, d, axis)
            assert not (need & ~real[src]).any(), (it, name, src)
        # blend reads covered
        for r, segs in p.blend.items():
            lvl, roff, coff = BLEND_SRC[r]
            for (_k, rlo, rhi, clo, chi) in segs:
                ra, rb = rlo + roff - p.OB, rhi + roff - p.OB
                sub = real[lvl][ra:rb, clo + coff:chi + coff]
                assert sub.all(), (it, r, _k)
        # carry feasibility: this band's carry req ⊆ prev band's realized
        if it > 0:
            prev = PLANS[it - 1]
            pr = np.zeros((CANV_R, CANV_C), bool)
            _paint(pr, prev.ee_groups)
            if it - 1 == 0:
                pr[0:14] = True
            assert not (p.carry_ee & ~pr[32:46]).any(), (it, "carry_ee")
            po = np.zeros((CANV_R, CANV_C), bool)
            _paint(po, prev.oo_groups)
            if prev.oo13 is not None:
                po[13, prev.oo13[0]:prev.oo13[1]] = True
            assert not (p.carry_oo & ~po[32:46]).any(), (it, "carry_oo")
            for name, lo in PERSIST.items():
                cn = p.carry_pers_out.get(name)
                if cn is None or not cn.any():
                    continue
                pm = np.zeros((CANV_R, CANV_C), bool)
                _paint(pm, prev.groups.get(name, []))
                assert not (cn & ~pm[32:lo + 32]).any(), (it, name)


_validate_plans()

# ---- tile sizing ----

LV_TAG = dict(a2="tP", a4="tP", a8="tP", v13="tP",
              s2="tQ", s8="tQ", s4="tS4", u7="tT", w10="tT",
              s7="tS7", a2o="tPo", a4o="tPo", a8o="tPo",
              s2o="tQo", s8o="tQo", s4o="tS4o", s10="tS10")


def _bbox(rects):
    r0 = min(a for a, b, c, d in rects)
    r1 = max(b for a, b, c, d in rects)
    c0 = min(c for a, b, c, d in rects)
    c1 = max(d for a, b, c, d in rects)
    return r0, r1, c0, c1


LV_ORIGIN = []        # per band: level -> (r0, c0) canvas origin of tile
TAG_MAX = {}
for _p in PLANS:
    org = {}
    for _n in LEVELS:
        rects = _p.groups.get(_n, [])
        if not rects:
            continue
        r0, r1, c0, c1 = _bbox(rects)
        org[_n] = (r0, c0)
        if _n == "s13" or _n in PERSIST:
            continue
        t = LV_TAG[_n]
        sz = TAG_MAX.get(t, (0, 0))
        TAG_MAX[t] = (max(sz[0], r1 - r0), max(sz[1], c1 - c0))
    LV_ORIGIN.append(org)

PERS_DIM = {}
for _n, _lo in PERSIST.items():
    _w = 1
    for _p in PLANS:
        for (_a, _b, _c, _d) in _p.groups.get(_n, []):
            _w = max(_w, _d)
    PERS_DIM[_n] = (_lo + 32, _w)

# ---- blend mask strips ----


def _strip_mask(it, rlo, rhi, clo, chi, ring):
    y0 = PLANS[it].y0
    n0 = (_R0 == ring)[rlo:rhi, clo:chi].astype(np.uint8)
    n1 = (_R1[:, 0:CEN] if False else (_R1 == ring))[rlo:rhi, clo:chi] \
        .astype(np.uint8)
    m = np.zeros((128, rhi - rlo, chi - clo), np.uint8)
    m[0:64] = n0[None]
    m[64:128] = n1[None]
    return m


def _build_strips():
    blobs, bands, offs = [], [], []
    pos = 0
    for it, p in enumerate(PLANS):
        start = pos
        ent = {}
        for r in (3, 2, 1, 0):
            for gi, seg in enumerate(p.blend.get(r, [])):
                kind, rlo, rhi, clo, chi = seg
                if kind != "cp":
                    continue
                m = _strip_mask(it, rlo, rhi, clo, chi, r)
                nr, nc = m.shape[1], m.shape[2]
                ent[(r, gi)] = (pos - start, nr, nc)
                blobs.append(np.ascontiguousarray(m).reshape(128, -1))
                pos += nr * nc
        offs.append(ent)
        bands.append((start, pos - start))
    blob = (np.concatenate(blobs, 1) if blobs
            else np.zeros((128, 1), np.uint8))
    return blob, bands, offs


STRIP_BLOB, STRIP_BANDS, STRIP_OFFS = _build_strips()
STRIP_MAX = max(max(sz for _, sz in STRIP_BANDS), 1)

# ---------------- shared band program ----------------


def _emit_band(be, it):
    p = PLANS[it]
    y0, y1, H, OB = p.y0, p.y1, p.H, p.OB
    ee, oo, Ew, ol = be.ee, be.oo, be.Ew, be.owlast

    # ---- input DMA ----
    if 0 < it < 7:
        az = be.azeo()
        be.dma_band(it, az)
    elif it == 0:
        az = be.azeo()
        for r0, r1 in ((0, 4), (4, 8), (8, 16), (16, 24), (24, 32),
                       (32, 40), (40, 48), (48, 64)):
            be.dma_rows(az, r0, r1)

    # ---- Ew build ----
    if it < 7:
        for (a, b, c, d) in p.ew_groups:
            be.max2(Ew[:, a:b, c:d], az[:, a:b, c:d], az[:, a:b, 126 + c:126 + d])

    # ---- carries ----
    if it > 0:
        for (c0, c1) in p.carry_ee_copy:
            be.gcopy(ee[:, 0:14, c0:c1], ee[:, 32:46, c0:c1])
        for (c0, c1) in p.carry_oo_copy:
            be.gcopy(oo[:, 0:14, c0:c1], oo[:, 32:46, c0:c1])
        for name in PERSIST:
            cc = p.carry_pers_copy.get(name)
            if cc is None:
                continue
            ra, rb, ivs = cc
            t = be.pers_lv(name)
            for (c0, c1) in ivs:
                be.gcopy(t[:, ra:rb, c0:c1], t[:, ra + 32:rb + 32, c0:c1])
    else:
        be.memset(ee[:, 0:14, :], NEG)
        be.memset(oo[:, 0:14, :], NEG)

    # ---- ee/oo fresh ----
    if it == 7:
        be.memset(ee[:, 14:46, :], NEG)
        be.memset(oo[:, 14:46, :], NEG)
        if p.oo13 is not None:
            s0, s1 = p.oo13
            be.scopy(oo[:, 13:14, s0:s1], ol[:, 0:1, s0:s1])
    else:
        for (a, b, c, d) in p.ee_groups:
            j0, j1 = a - 14, b - 14
            be.max2(ee[:, a:b, c:d], Ew[:, 2 * j0:2 * j1:2, c:d],
                    Ew[:, 2 * j0 + 1:2 * j1:2, c:d])
        if p.oo13 is not None:
            s0, s1 = p.oo13
            be.max2(oo[:, 13:14, s0:s1], az[:, 0:1, 126 + s0:126 + s1],
                    az[:, 0:1, 1 + s0:1 + s1])
            if it > 0:
                be.max2(oo[:, 13:14, s0:s1], oo[:, 13:14, s0:s1],
                        ol[:, 0:1, s0:s1])
        for (a, b, c, d) in p.oo_groups:
            j0, j1 = a - 14, b - 14
            be.max2(oo[:, a:b, c:d],
                    az[:, 2 * j0 + 1:2 * j1 + 1:2, 126 + c:126 + d],
                    az[:, 2 * j0 + 1:2 * j1 + 1:2, 1 + c:1 + d])
            be.max2(oo[:, a:b, c:d], oo[:, a:b, c:d],
                    az[:, 2 * j0 + 2:2 * j1 + 2:2, 126 + c:126 + d])
            be.max2(oo[:, a:b, c:d], oo[:, a:b, c:d],
                    az[:, 2 * j0 + 2:2 * j1 + 2:2, 1 + c:1 + d])
        if p.owlast_cols is not None:
            g0, g1 = p.owlast_cols
            be.max2(ol[:, 0:1, g0:g1], az[:, 63:64, 126 + g0:126 + g1],
                    az[:, 63:64, 1 + g0:1 + g1])

    # ---- pyramid ----
    out16 = be.out16()
    tiles = {"ee": (ee, 0, 0), "oo": (oo, 0, 0)}
    for _n in PERSIST:
        tiles[_n] = (be.pers_lv(_n), 0, 0)
    org = LV_ORIGIN[it]

    def src_ap(st, sr0, sc0, a, b, c, d):
        return st[:, a - sr0:b - sr0, c - sc0:d - sc0]

    emit_order = ["a2", "a2o", "s2", "s2o", "a4", "a4o", "s4", "s4o",
                  "a8", "a8o", "u7", "s8", "s8o", "s7", "w10", "v13",
                  "s10", "s13"]
    for name in emit_order:
        rects = p.groups.get(name, [])
        if not rects:
            continue
        srcn, dsh, axis = LEVELS[name]
        st, sr0, sc0 = tiles[srcn]
        if name == "s13":
            # write directly into out16: out row = canvas row + OB + 6
            for (a, b, c, d) in rects:
                ra, rb = a + OB + 6 - y0, b + OB + 6 - y0
                dst = out16[:, ra:rb, c:d]
                be.max2(dst, src_ap(st, sr0, sc0, a, b, c, d),
                        src_ap(st, sr0, sc0, a, b, c + dsh, d + dsh))
            continue
        if name in PERSIST:
            t = be.pers_lv(name)
            r0 = c0 = 0
        else:
            r0, c0 = org[name]
            t = be.lv(name)
        tiles[name] = (t, r0, c0)
        for (a, b, c, d) in rects:
            dst = t[:, a - r0:b - r0, c - c0:d - c0]
            if axis == "r":
                be.max2(dst, src_ap(st, sr0, sc0, a, b, c, d),
                        src_ap(st, sr0, sc0, a + dsh, b + dsh, c, d))
            else:
                be.max2(dst, src_ap(st, sr0, sc0, a, b, c, d),
                        src_ap(st, sr0, sc0, a, b, c + dsh, d + dsh))

    # ---- blends (rings 3,2,1,0 after s13 write) ----
    if any(k[0] in (0, 1, 2, 3) for k in STRIP_OFFS[it]):
        be.dma_strip(it)
    for r in (3, 2, 1, 0):
        lvl, roff, coff = BLEND_SRC[r]
        st, sr0, sc0 = tiles.get(lvl, (None, 0, 0))
        for gi, seg in enumerate(p.blend.get(r, [])):
            kind, rlo, rhi, clo, chi = seg
            ra, rb = rlo + roff - OB, rhi + roff - OB
            data = st[:, ra - sr0:rb - sr0,
                      clo + coff - sc0:chi + coff - sc0]
            dst = out16[:, rlo - y0:rhi - y0, clo:chi]
            if kind == "cp":
                be.cp(dst, be.strip_ap(it, (r, gi)), data)
            else:
                be.acopy(dst, data)

    be.dma_out(it, out16)


def _emit_program(be):
    for it in range(NBANDS):
        _emit_band(be, it)


def prep_input(x1):
    """x1 [C, 448, 448] fp32 -> [128, 448, 250] fp16 parity-split blob.
    Per (wg*64+c, row): [pad6 | zE (119) | pad7 | zO (118)], wg1 mirrored,
    NEG pads.  Pure layout marshalling (cast/reorder/pad), no arithmetic."""
    xz = np.full((2, C, IN, 250), NEG, np.float16)
    xz[0, :, :, 6:125] = x1[:, :, 0:237:2]       # zE0[e]=x[2e-12]
    xz[0, :, :, 132:250] = x1[:, :, 1:236:2]     # zO0[e]=x[2e-11]
    xz[1, :, :, 6:125] = x1[:, :, 447:209:-2]    # zE1[e]=x[459-2e]
    xz[1, :, :, 132:250] = x1[:, :, 446:210:-2]  # zO1[e]=x[458-2e]
    return np.ascontiguousarray(xz.reshape(128, IN, 250))


# ---------------- numpy backend (validation) ----------------


class NumpyBE:
    def __init__(self, x):
        self.xz = prep_input(x).astype(np.float32)
        f32 = np.float32
        self._azeo = np.full((128, 64, 250), np.nan, f32)
        self.Ew = np.full((128, 64, EOW), np.nan, f32)
        self.owlast = np.full((128, 1, EOW), np.nan, f32)
        self.ee = np.full((128, 46, EOW), np.nan, f32)
        self.oo = np.full((128, 46, EOW), np.nan, f32)
        self._pers = {n: np.full((128, r, w), np.nan, f32)
                      for n, (r, w) in PERS_DIM.items()}
        self.y = np.full((128, OUT, OW), np.nan, f32)
        self._out = None

    def azeo(self):
        return self._azeo

    def lv(self, name):
        nr, nc = TAG_MAX[LV_TAG[name]]
        return np.full((128, nr, nc), np.nan, np.float32)

    def pers_lv(self, name):
        return self._pers[name]

    def out16(self):
        self._out = np.full((128, 32, OW), np.nan, np.float32)
        return self._out

    def memset(self, ap, v):
        ap[...] = v

    def max2(self, d, a, b):
        assert d.shape == a.shape == b.shape, (d.shape, a.shape, b.shape)
        np.maximum(a, b, out=d)

    def scopy(self, d, s):
        d[...] = s

    acopy = scopy
    gcopy = scopy

    def cp(self, out, mask, data):
        assert out.shape == mask.shape == data.shape
        out[...] = np.where(mask != 0, data, out)

    def dma_band(self, it, az):
        r0 = 64 * it
        az[:, :, :] = self.xz[:, r0:r0 + 64, :]

    def dma_rows(self, az, a, b):
        az[:, a:b, :] = self.xz[:, a:b, :]

    def dma_strip(self, it):
        pass

    def strip_ap(self, it, key):
        start, _ = STRIP_BANDS[it]
        off, nr, nc = STRIP_OFFS[it][key]
        return STRIP_BLOB[:, start + off:start + off + nr * nc].reshape(
            128, nr, nc)

    def dma_out(self, it, out16):
        p = PLANS[it]
        self.y[:, p.y0:p.y1, :] = out16[:, 0:p.H, :]


def numpy_kernel(x1):
    """x1: [64, 448, 448] -> [64, 224, 224] (fp32, exact clip semantics)."""
    be = NumpyBE(x1)
    _emit_program(be)
    yw = be.y
    assert not np.isnan(yw).any(), "uncovered output pixels"
    out = np.empty((C, OUT, OUT), np.float32)
    out[:, :, 0:OW] = yw[0:64]
    out[:, :, OW:OUT] = yw[64:128, :, ::-1]
    return out


# ---------------- bass backend ----------------


def split_multi_waits(nc):
    """walrus CoreV3Gen accepts at most 1 sync-wait per instruction; Tile's
    tail drains can carry 2+.  Peel extras onto preceding NoOps."""
    n = 0
    for fn in nc.m.functions:
        for bb in fn.blocks:
            insts = list(bb.instructions)
            out = []
            for ins in insts:
                si = getattr(ins, "sync_info", None)
                if si is not None and len(si.on_wait) > 1:
                    waits = list(si.on_wait)
                    for k, w in enumerate(waits[:-1]):
                        nop = mybir.InstNoOp(
                            name=f"{ins.name}-waitsplit{k}",
                            engine=ins.engine, ins=[], outs=[])
                        nop.sync_info = mybir.SyncInfo(
                            on_wait=[w], on_update=[])
                        out.append(nop)
                        n += 1
                    ins.sync_info = mybir.SyncInfo(
                        on_wait=[waits[-1]], on_update=list(si.on_update))
                out.append(ins)
            if n:
                bb.instructions = out
    return n


class BassBE:
    def __init__(self, nc, pools, x, y, strips):
        self.nc = nc
        self.x = x
        self.y = y
        self.strips = strips
        pers, self.lvpool, self.iop, self.chpool, self.strippool = pools
        self.Ew = pers.tile([128, 64, EOW], DT, tag="Ew")
        self.owlast = pers.tile([128, 1, EOW], DT, tag="owlast")
        self.ee = pers.tile([128, 46, EOW], DT, tag="ee")
        self.oo = pers.tile([128, 46, EOW], DT, tag="oo")
        self.pers_tiles = {
            n: pers.tile([128, r, w], DT, tag=f"pers_{n}", name=f"pers_{n}")
            for n, (r, w) in PERS_DIM.items()}
        self._strip = None

    def azeo(self):
        return self.chpool.tile([128, 64, 250], DT, tag="azeo", name="azeo")

    def lv(self, name):
        nr, nc_ = TAG_MAX[LV_TAG[name]]
        return self.lvpool.tile([128, nr, nc_], DT, tag=LV_TAG[name],
                                name=f"lv_{name}")

    def pers_lv(self, name):
        return self.pers_tiles[name]

    def out16(self):
        return self.iop.tile([128, 32, OW], DT, tag="out16", name="out16")

    def memset(self, ap, v):
        self.nc.gpsimd.memset(ap, v)

    def max2(self, d, a, b):
        self.nc.vector.tensor_tensor(d, a, b, MX)

    def scopy(self, d, s):
        self.nc.vector.tensor_scalar_max(d, s, NEG)

    def acopy(self, d, s):
        self.nc.scalar.copy(d, s)

    gcopy = acopy

    def cp(self, out, mask, data):
        self.nc.vector.copy_predicated(out, mask, data)

    def dma_band(self, it, az):
        r0 = 64 * it
        self.nc.sync.dma_start(az[:, :, :], self.x[:, r0:r0 + 64, :])

    def dma_rows(self, az, a, b):
        self.nc.sync.dma_start(az[:, a:b, :], self.x[:, a:b, :])

    def dma_strip(self, it):
        start, sz = STRIP_BANDS[it]
        self._strip = self.strippool.tile([128, STRIP_MAX], mybir.dt.uint8,
                                          tag="strip", name="strip")
        self.nc.sync.dma_start(self._strip[:, 0:sz],
                               self.strips[:, start:start + sz])

    def strip_ap(self, it, key):
        off, nr, nc_ = STRIP_OFFS[it][key]
        return self._strip[:, off:off + nr * nc_].rearrange(
            "p (r c) -> p r c", c=nc_)

    def dma_out(self, it, out16):
        p = PLANS[it]
        self.nc.sync.dma_start(self.y[:, p.y0:p.y1, :],
                               out16[:, 0:p.H, :])


def _emit_kernel(nc: bass.Bass):
    x = nc.dram_tensor("x", [128, IN, 250], DT, kind="ExternalInput")
    y = nc.dram_tensor("y", [128, OUT, OW], DT, kind="ExternalOutput")
    strips = nc.inline_tensor(STRIP_BLOB, name="mstrips")

    with TileContext(nc) as tc:
        with tc.tile_pool(name="pp", bufs=1) as pers, \
             tc.tile_pool(name="lv", bufs=1) as lvpool, \
             tc.tile_pool(name="io", bufs=1) as iop, \
             tc.tile_pool(name="ch", bufs=2) as chpool, \
             tc.tile_pool(name="st", bufs=1) as strippool:
            be = BassBE(nc, (pers, lvpool, iop, chpool, strippool), x, y, strips)
            _emit_program(be)
    return nc


_CACHED = {}


def _get_nc():
    if "nc" not in _CACHED:
        nc = bass.Bass()
        _emit_kernel(nc)
        split_multi_waits(nc)
        _CACHED["nc"] = nc
    return _CACHED["nc"]


def kernel(x: np.ndarray) -> np.ndarray:
    nc = _get_nc()
    in_maps = [{"x": prep_input(x[b].astype(np.float32))}
               for b in range(B)]
    res = run_bass_kernel_spmd(nc, in_maps, core_ids=list(range(B)))
    out = np.empty((B, C, OUT, OUT), np.float32)
    for b, r in enumerate(res.results):
        yw = r["y"].astype(np.float32)      # [128, 224, 112]
        out[b, :, :, 0:OW] = yw[0:64]
        out[b, :, :, OW:OUT] = yw[64:128, :, ::-1]
    return out
